# revision 1
# baseline (speedup 1.0000x reference)
"""Trainium2 Bass kernel for nn_DecoderMinLSTMGNN.

Model (per sample): two MinLSTM layers (D=512) over T=4096 steps, residual,
LayerNorm, projection D->1.  B=8 samples are data-parallel across the 8
NeuronCores (one sample per core).

Per-core layout is channels-major: x^T [D, T] so the time-dim linear
recurrence h_t = a_t*h_{t-1} + (1-a_t)*htilde_t maps onto the VectorE
TensorTensorScan instruction (scan along the free dim, 128 channels per
partition group, carried across 8 time tiles of 512).

Gate math per (group g, time-tile t):
  zf, zi, zh accumulate in PSUM over 4 k-chunks of fp32r matmuls;
  zh additionally gets its bias via a k=1 matmul (bias row x ones row)
  f = sigmoid(zf + bf), i = sigmoid(zi + bi)   (ScalarE, "sigmoid" act set)
  den = f + i                                  (VectorE)
  r = 1/den                                    (ScalarE Reciprocal LUT,
                                                "reciprocal" act set)
  a = f * r                                    (VectorE)
  u' = (a - 1) * zh_psum                       (VectorE scalar_tensor_tensor)
  h  = scan: state = a*state - u'              (VectorE tensor_tensor_scan)
The two ScalarE act-table switches per (t, layer) block cost ~1.3us each.
The VectorE reciprocal instruction is NOT used in the hot loop (it runs at
~6 cycles/elem); the ScalarE Reciprocal LUT runs at streaming rate.

Epilogue: res = h2 + x^T; LN stats + output projection are matmul
reductions against [ones | W_out*ln_g] accumulated into persistent PSUM
banks (partition index = time tile), final LN math is batched on [8,512]
tiles, output DMA'd as [8,512] -> y[4096].
"""

import numpy as np

import concourse.bass as bass
import concourse.mybir as mybir
import concourse.tile as tile
from concourse.bass_utils import run_bass_kernel_spmd

F32 = mybir.dt.float32
F32R = mybir.dt.float32r
AF = mybir.ActivationFunctionType
OP = mybir.AluOpType

B, T, D = 8, 4096, 512
OUT = 1
LN_EPS = 1e-5
TT = 512                 # time-tile size
NT = T // TT             # 8 time tiles
G = D // 128             # 4 channel groups
K = D // 128             # 4 contraction chunks

MAX_WAITS = 1


def _split_excess_waits(nc):
    """walrus in this container rejects >1 semaphore wait per instruction
    ("Too many sync wait commands"); move excess waits onto NoOps."""
    for fn in nc.m.functions:
        for bb in fn.blocks:
            new_list = []
            changed = False
            for inst in bb.instructions:
                si = inst.sync_info
                waits = list(si.on_wait) if si is not None and si.on_wait else []
                if len(waits) > MAX_WAITS:
                    changed = True
                    overflow = waits[:-MAX_WAITS]
                    si.on_wait = waits[-MAX_WAITS:]
                    for j in range(0, len(overflow), MAX_WAITS):
                        new_list.append(mybir.InstNoOp(
                            name=f"{inst.name}-waitsplit-{j}",
                            engine=inst.engine,
                            ins=[], outs=[],
                            sync_info=mybir.SyncInfo(
                                on_wait=overflow[j:j + MAX_WAITS], on_update=[]),
                        ))
                new_list.append(inst)
            if changed:
                bb.instructions[:] = new_list
    return nc


def _act_direct(nc, out, in_, func, bias=0.0, scale=1.0):
    """emit InstActivation directly (bass blocks Reciprocal/Rsqrt)."""
    ins = [nc.scalar.lower_ap(in_)]
    for v in (bias, scale, 0.0):
        if isinstance(v, (int, float)):
            ins.append(mybir.ImmediateValue(dtype=mybir.dt.float32, value=float(v)))
        else:
            ins.append(nc.scalar.lower_ap(v))
    return nc.scalar.add_instruction(
        mybir.InstActivation(
            name=nc.get_next_instruction_name(),
            func=func, ins=ins, outs=[nc.scalar.lower_ap(out)]))


def _build_nc():
    nc = bass.Bass()

    xt_d = nc.dram_tensor("xt", [D, T], F32R, kind="ExternalInput")
    wt_d = nc.dram_tensor("wt", [6, D, D], F32R, kind="ExternalInput")
    # f/i biases per layer: bias[p, layer, {f,i}, g] = b[g*128+p]
    bias_d = nc.dram_tensor("bias", [128, 2, 2, G], F32, kind="ExternalInput")
    # h-gate bias rows (layer, g) -> [1, 128], matmul'd against a ones row
    brow_d = nc.dram_tensor("brow", [2 * G, 128], F32R, kind="ExternalInput")
    ones_d = nc.dram_tensor("ones", [1, TT], F32R, kind="ExternalInput")
    # stats lhsT per (g,t): col t = 1, col 32+t = wg[g*128:(g+1)*128]
    slt_d = nc.dram_tensor("slt", [G, NT, 128, 40], F32R, kind="ExternalInput")
    # S2 lhsT per t: col t = 1
    s2l_d = nc.dram_tensor("s2l", [NT, 128, NT], F32R, kind="ExternalInput")
    epi_d = nc.dram_tensor("epi", [NT, 3], F32, kind="ExternalInput")  # [c0, swg/D, eps]
    out_d = nc.dram_tensor("out", [NT, TT], F32, kind="ExternalOutput")

    with tile.TileContext(nc) as tc:
        with (
            tc.tile_pool(name="const", bufs=1) as const,
            tc.tile_pool(name="xtp", bufs=1) as xtp,
            tc.tile_pool(name="work", bufs=2) as work,
            tc.tile_pool(name="hpool", bufs=2) as hpool,
            tc.tile_pool(name="fin", bufs=1) as fin,
            tc.tile_pool(name="gates_ps", bufs=2, space="PSUM") as gates_ps,
            tc.tile_pool(name="stats_ps", bufs=1, space="PSUM") as stats_ps,
        ):
            # ---- constants ----
            wt_sb = []
            for idx in range(6):
                w = const.tile([128, K, D], F32R, tag=f"wt{idx}")
                nc.sync.dma_start(
                    out=w[:], in_=wt_d[idx].rearrange("(k p) d -> p k d", p=128))
                wt_sb.append(w)
            bias_sb = const.tile([128, 2, 2, G], F32)
            nc.sync.dma_start(out=bias_sb[:], in_=bias_d[:])
            brow_sb = const.tile([1, 2 * G, 128], F32R)
            nc.sync.dma_start(out=brow_sb[:], in_=brow_d[None, :, :])
            ones_sb = const.tile([1, TT], F32R)
            nc.sync.dma_start(out=ones_sb[:], in_=ones_d[:])
            slt_sb = const.tile([128, G, NT, 40], F32R)
            nc.sync.dma_start(
                out=slt_sb[:], in_=slt_d.rearrange("g t p c -> p g t c"))
            s2l_sb = const.tile([128, NT, NT], F32R)
            nc.sync.dma_start(out=s2l_sb[:], in_=s2l_d.rearrange("t p c -> p t c"))
            epi_sb = const.tile([NT, 3], F32)
            nc.sync.dma_start(out=epi_sb[:], in_=epi_d[:])

            # ---- x^T resident tiles, one DMA per (k, t) ----
            xt_sb = [[None] * NT for _ in range(K)]
            for k in range(K):
                for t in range(NT):
                    xx = xtp.tile([128, TT], F32R, tag=f"xt{k}_{t}")
                    nc.sync.dma_start(
                        out=xx[:],
                        in_=xt_d[k * 128:(k + 1) * 128, t * TT:(t + 1) * TT])
                    xt_sb[k][t] = xx

            # persistent stats accumulators (PSUM)
            s13_ps = stats_ps.tile([40, TT], F32, tag="s13")
            s2_ps = stats_ps.tile([NT, TT], F32, tag="s2")
            stats_first = [True]

            h1_sb = [[None] * NT for _ in range(G)]   # layer-1 outputs (F32R)
            h2_sb = [[None] * NT for _ in range(G)]   # layer-2 outputs (F32)

            def layer_tile(layer, t):
                """emit one time-tile of one MinLSTM layer (all 4 groups)"""
                rhs = (xt_sb if layer == 0 else h1_sb)
                h_out = (h1_sb if layer == 0 else h2_sb)
                h_dtype = F32R if layer == 0 else F32
                widx0 = 3 * layer

                pf_l, pi_l, ph_l = [], [], []
                for g in range(G):
                    pf = gates_ps.tile([128, TT], F32, tag="pf")
                    pi = gates_ps.tile([128, TT], F32, tag="pi")
                    ph = gates_ps.tile([128, TT], F32, tag="ph")
                    for gate, ps in ((0, pf), (1, pi), (2, ph)):
                        w = wt_sb[widx0 + gate]
                        for k in range(K):
                            r = rhs[k][t]
                            nc.tensor.matmul(
                                ps[:],
                                w[:, k, g * 128:(g + 1) * 128],
                                r[:] if layer == 0 else r[:].bitcast(F32R),
                                start=(k == 0),
                                stop=(k == K - 1) and (ps is not ph))
                    # h-gate bias via k=1 matmul: ph += bh_row x ones
                    nc.tensor.matmul(
                        ph[:], brow_sb[:, layer * G + g, :], ones_sb[:],
                        start=False, stop=True)
                    pf_l.append(pf)
                    pi_l.append(pi)
                    ph_l.append(ph)

                # phase 1 (sigmoid table): f, i for all groups
                f_l, i_l = [], []
                for g in range(G):
                    f_sb = work.tile([128, TT], F32, tag="f")
                    nc.scalar.activation(
                        f_sb[:], pf_l[g][:], AF.Sigmoid,
                        bias=bias_sb[:, layer, 0, g:g + 1])
                    i_sb = work.tile([128, TT], F32, tag="i")
                    nc.scalar.activation(
                        i_sb[:], pi_l[g][:], AF.Sigmoid,
                        bias=bias_sb[:, layer, 1, g:g + 1])
                    f_l.append(f_sb)
                    i_l.append(i_sb)

                # den = f + i (DVE), then phase 2 (reciprocal table): r = 1/den
                den_l = []
                for g in range(G):
                    den_sb = work.tile([128, TT], F32, tag="den")
                    nc.vector.tensor_add(den_sb[:], f_l[g][:], i_l[g][:])
                    den_l.append(den_sb)
                for g in range(G):
                    r_sb = work.tile([128, TT], F32, tag="r")
                    _act_direct(nc, r_sb[:], den_l[g][:], AF.Reciprocal)
                    a_sb = work.tile([128, TT], F32, tag="a")
                    nc.vector.tensor_mul(a_sb[:], f_l[g][:], r_sb[:])
                    # u' = (a - 1) * zh   (zh read straight from PSUM)
                    up_sb = work.tile([128, TT], F32, tag="up")
                    nc.vector.scalar_tensor_tensor(
                        up_sb[:], a_sb[:], 1.0, ph_l[g][:], OP.subtract, OP.mult)
                    h_sb = hpool.tile([128, TT], h_dtype, tag=f"h{layer}_{g}")
                    init = 0.0 if t == 0 else h_out[g][t - 1][:, TT - 1:TT]
                    nc.vector.tensor_tensor_scan(
                        h_sb[:], a_sb[:], up_sb[:], init, OP.mult, OP.subtract)
                    h_out[g][t] = h_sb

            def epilogue_tile(t):
                """residual + LN/output stats for one time tile"""
                for g in range(G):
                    res = work.tile([128, TT], F32R, tag="res")
                    nc.vector.tensor_add(
                        res[:], h2_sb[g][t][:], xt_sb[g][t][:].bitcast(F32))
                    sq = work.tile([128, TT], F32R, tag="sq")
                    nc.scalar.activation(sq[:], res[:].bitcast(F32), AF.Square)
                    first = stats_first[0]
                    stats_first[0] = False
                    last = (t == NT - 1 and g == G - 1)
                    nc.tensor.matmul(
                        s13_ps[:], slt_sb[:, g, t, :], res[:],
                        start=first, stop=last, skip_group_check=True)
                    nc.tensor.matmul(
                        s2_ps[:], s2l_sb[:, t, :], sq[:],
                        start=first, stop=last, skip_group_check=True)

            # ---- pipeline ----
            for t in range(NT):
                layer_tile(0, t)
                if t >= 1:
                    layer_tile(1, t - 1)
                if t >= 2:
                    epilogue_tile(t - 2)
            layer_tile(1, NT - 1)
            epilogue_tile(NT - 2)
            epilogue_tile(NT - 1)

            # ---- final LN + projection math on [8, 512] ----
            s1 = s13_ps[0:NT, :]
            s3p = s13_ps[32:32 + NT, :]
            s3_sb = fin.tile([NT, TT], F32, tag="s3f")
            nc.scalar.activation(s3_sb[:], s3p, AF.Copy)
            # nn = (s1 * swg/D) - s3
            nn_sb = fin.tile([NT, TT], F32, tag="nn")
            nc.vector.scalar_tensor_tensor(
                nn_sb[:], s1, epi_sb[:, 1:2], s3_sb[:], OP.mult, OP.subtract)
            # s1sq = (s1/D)^2
            s1sq_sb = fin.tile([NT, TT], F32, tag="s1sq")
            nc.scalar.activation(s1sq_sb[:], s1, AF.Square, scale=1.0 / D)
            # v = s2/D - s1sq
            v_sb = fin.tile([NT, TT], F32, tag="v")
            nc.vector.scalar_tensor_tensor(
                v_sb[:], s2_ps[:], 1.0 / D, s1sq_sb[:], OP.mult, OP.subtract)
            # rv = rsqrt(v + eps)  (one more act-table switch, at the very end)
            rv_sb = fin.tile([NT, TT], F32, tag="rv")
            _act_direct(nc, rv_sb[:], v_sb[:], AF.Rsqrt, bias=epi_sb[:, 2:3])
            # pr = (nn * -1) * rv = (s3 - mu*swg) * rv
            pr_sb = fin.tile([NT, TT], F32, tag="pr")
            nc.vector.scalar_tensor_tensor(
                pr_sb[:], nn_sb[:], -1.0, rv_sb[:], OP.mult, OP.mult)
            # out = pr + c0
            o_sb = fin.tile([NT, TT], F32, tag="o")
            nc.scalar.activation(o_sb[:], pr_sb[:], AF.Identity,
                                 bias=epi_sb[:, 0:1])
            nc.sync.dma_start(out=out_d[:], in_=o_sb[:])

    _split_excess_waits(nc)
    return nc


_NC_CACHE = None


def _get_nc():
    global _NC_CACHE
    if _NC_CACHE is None:
        _NC_CACHE = _build_nc()
    return _NC_CACHE


def _host_prep(inputs):
    x = np.asarray(inputs["x"], dtype=np.float32)
    Ws = [inputs[n] for n in ("Wf0", "Wi0", "Wh0", "Wf1", "Wi1", "Wh1")]
    bs = [np.asarray(inputs[n], np.float32) for n in
          ("bf0", "bi0", "bh0", "bf1", "bi1", "bh1")]
    wt_all = np.ascontiguousarray(
        np.stack([np.asarray(w, np.float32).T for w in Ws]))      # [6, din, dout]
    # f/i biases: bias[p, layer, {f,i}, g] = b[g*128+p]
    bias_all = np.zeros((128, 2, 2, G), np.float32)
    for layer in range(2):
        for j in range(2):
            bias_all[:, layer, j, :] = bs[3 * layer + j].reshape(G, 128).T
    # h-gate bias rows: brow[layer*G+g, c] = bh[g*128+c]
    brow = np.zeros((2 * G, 128), np.float32)
    for layer in range(2):
        brow[layer * G:(layer + 1) * G] = bs[3 * layer + 2].reshape(G, 128)
    ones = np.ones((1, TT), np.float32)

    w_out = np.asarray(inputs["W_out"], np.float32).reshape(D)
    ln_g = np.asarray(inputs["ln_g"], np.float32)
    ln_b = np.asarray(inputs["ln_b"], np.float32)
    b_out = np.asarray(inputs["b_out"], np.float32).reshape(())
    wg = w_out * ln_g
    c0 = float(np.dot(w_out, ln_b) + b_out)
    swg = float(wg.sum())

    slt = np.zeros((G, NT, 128, 40), np.float32)
    for g in range(G):
        for t in range(NT):
            slt[g, t, :, t] = 1.0
            slt[g, t, :, 32 + t] = wg[g * 128:(g + 1) * 128]
    s2l = np.zeros((NT, 128, NT), np.float32)
    for t in range(NT):
        s2l[t, :, t] = 1.0
    epi = np.zeros((NT, 3), np.float32)
    epi[:, 0] = c0
    epi[:, 1] = swg / D
    epi[:, 2] = LN_EPS
    return x, wt_all, bias_all, brow, ones, slt, s2l, epi


def _in_maps(inputs):
    x, wt_all, bias_all, brow, ones, slt, s2l, epi = _host_prep(inputs)
    return [
        {
            "xt": np.ascontiguousarray(x[b].T),
            "wt": wt_all, "bias": bias_all, "brow": brow, "ones": ones,
            "slt": slt, "s2l": s2l, "epi": epi,
        }
        for b in range(B)
    ]


def kernel(**inputs):
    nc = _get_nc()
    res = run_bass_kernel_spmd(nc, _in_maps(inputs), list(range(B)))
    out = np.stack([res.results[b]["out"].reshape(T, OUT) for b in range(B)])
    return out.astype(np.float32)


def kernel_traced(**inputs):
    """same as kernel() but returns (output, BassKernelResults) with timing"""
    nc = _get_nc()
    res = run_bass_kernel_spmd(nc, _in_maps(inputs), list(range(B)), trace=True)
    out = np.stack([res.results[b]["out"].reshape(T, OUT) for b in range(B)])
    return out.astype(np.float32), res



# revision 6
# speedup vs baseline: 1.0621x; 1.0621x over previous
"""Trainium2 Bass kernel for nn_DecoderMinLSTMGNN.

Model (per sample): two MinLSTM layers (D=512) over T=4096 steps, residual,
LayerNorm, projection D->1.  B=8 samples are data-parallel across the 8
NeuronCores (one sample per core).

Per-core layout is channels-major: x^T [D, T], bf16.  The time-dim linear
recurrence h_t = a_t*h_{t-1} + (1-a_t)*htilde_t maps onto the VectorE
TensorTensorScan instruction (scan along the free dim).

Key optimizations over the naive structure:
- h-gate bias elimination: substitute g = h - bh.  The recurrence becomes
  bias-free (g_t = a_t g_{t-1} + (1-a_t) zh_t with zh = W_h x, init -bh);
  the bias reappears as a constant shift that folds into the next layer's
  f/i gate biases (bf1_eff = bf1 + Wf1 @ bh0) and into the LN/projection
  stats via an extra lhsT column + host-side constants.  Removes 64
  bias matmuls.
- ScalarE act-table phase batching: sigmoids and reciprocals use different
  act-table sets (~1.3us per table load; the naive interleaving costs ~97
  loads = 124us).  Work is organized in (layer, half-of-T) phases: all 32
  sigmoids of a phase, then all reciprocals.  A "min-gate" artificial
  dependency (phase biases are routed through tensor_tensor(min) against
  the previous phase's reciprocal output, exact because |bias|<0.45 and
  r=1/(f+i)>=0.5) forces the scheduler to keep phases contiguous:
  9 table loads total.
- bf16 operands everywhere (matmuls, DVE tensor-tensor 2x mode, half the
  SBUF/DMA footprint).  PSUM accumulation and scan state stay fp32.
- Wide [128, 2048] instructions for reciprocal/square/a/den/res/scan to
  amortize per-instruction overhead.
"""

import numpy as np
import ml_dtypes

import concourse.bass as bass
import concourse.mybir as mybir
import concourse.tile as tile
from concourse.bass_utils import run_bass_kernel_spmd

F32 = mybir.dt.float32
BF16 = mybir.dt.bfloat16
AF = mybir.ActivationFunctionType
OP = mybir.AluOpType

B, T, D = 8, 4096, 512
OUT = 1
LN_EPS = 1e-5
TT = 512                 # time-tile size
NT = T // TT             # 8 time tiles
G = D // 128             # 4 channel groups
K = D // 128             # 4 contraction chunks
TPH = 4                  # time tiles per phase (half)
HALF = TPH * TT          # 2048
NH = NT // TPH           # 2 halves

MAX_WAITS = 1


def _split_excess_waits(nc):
    """walrus in this container rejects >1 semaphore wait per instruction
    ("Too many sync wait commands"); move excess waits onto NoOps."""
    for fn in nc.m.functions:
        for bb in fn.blocks:
            new_list = []
            changed = False
            for inst in bb.instructions:
                si = inst.sync_info
                waits = list(si.on_wait) if si is not None and si.on_wait else []
                if len(waits) > MAX_WAITS:
                    changed = True
                    overflow = waits[:-MAX_WAITS]
                    si.on_wait = waits[-MAX_WAITS:]
                    for j in range(0, len(overflow), MAX_WAITS):
                        new_list.append(mybir.InstNoOp(
                            name=f"{inst.name}-waitsplit-{j}",
                            engine=inst.engine,
                            ins=[], outs=[],
                            sync_info=mybir.SyncInfo(
                                on_wait=overflow[j:j + MAX_WAITS], on_update=[]),
                        ))
                new_list.append(inst)
            if changed:
                bb.instructions[:] = new_list
    return nc


def _act_direct(nc, out, in_, func, bias=0.0, scale=1.0):
    """emit InstActivation directly (bass blocks Reciprocal/Rsqrt)."""
    ins = [nc.scalar.lower_ap(in_)]
    for v in (bias, scale, 0.0):
        if isinstance(v, (int, float)):
            ins.append(mybir.ImmediateValue(dtype=mybir.dt.float32, value=float(v)))
        else:
            ins.append(nc.scalar.lower_ap(v))
    return nc.scalar.add_instruction(
        mybir.InstActivation(
            name=nc.get_next_instruction_name(),
            func=func, ins=ins, outs=[nc.scalar.lower_ap(out)]))


def _build_nc():
    nc = bass.Bass()

    xt_d = nc.dram_tensor("xt", [D, T], BF16, kind="ExternalInput")
    wt_d = nc.dram_tensor("wt", [6, D, D], BF16, kind="ExternalInput")
    # f/i gate biases (layer-2 ones pre-corrected): [128, layer, gate*4+g]
    bias_d = nc.dram_tensor("bias", [128, 2, 8], F32, kind="ExternalInput")
    # scan initial state columns (-bh_eff): [128, layer, g]
    gi_d = nc.dram_tensor("gi", [128, 2, G], F32, kind="ExternalInput")
    # stats lhsT per (g,t): col t = 1 (s1), col 32+t = wg, col 64+t = c
    slt_d = nc.dram_tensor("slt", [G, NT, 128, 72], BF16, kind="ExternalInput")
    epi_d = nc.dram_tensor("epi", [NT, 8], F32, kind="ExternalInput")
    out_d = nc.dram_tensor("out", [NT, TT], F32, kind="ExternalOutput")

    with tile.TileContext(nc) as tc:
        with (
            tc.tile_pool(name="const", bufs=1) as const,
            tc.tile_pool(name="xtp", bufs=1) as xtp,
            tc.tile_pool(name="gp", bufs=1) as gp,        # wide per-g phase bufs
            tc.tile_pool(name="work", bufs=3) as work,    # i tiles
            tc.tile_pool(name="wk2", bufs=2) as wk2,      # res/sq/bgate/carry
            tc.tile_pool(name="fin", bufs=6) as fin,
            tc.tile_pool(name="gates_ps", bufs=2, space="PSUM") as gates_ps,
            tc.tile_pool(name="stats_ps", bufs=1, space="PSUM") as stats_ps,
        ):
            # ---- constants ----
            wt_sb = []
            for idx in range(6):
                w = const.tile([128, K, D], BF16, tag=f"wt{idx}")
                nc.sync.dma_start(
                    out=w[:], in_=wt_d[idx].rearrange("(k p) d -> p k d", p=128))
                wt_sb.append(w)
            bias_sb = const.tile([128, 2, 8], F32)
            nc.sync.dma_start(out=bias_sb[:], in_=bias_d[:])
            gi_sb = const.tile([128, 2, G], F32)
            nc.sync.dma_start(out=gi_sb[:], in_=gi_d[:])
            slt_sb = const.tile([128, G, NT, 72], BF16)
            nc.sync.dma_start(
                out=slt_sb[:], in_=slt_d.rearrange("g t p c -> p g t c"))
            epi_sb = const.tile([NT, 8], F32)
            nc.sync.dma_start(out=epi_sb[:], in_=epi_d[:])

            # ---- x^T resident, one DMA per (g, half) ----
            xt_sb = []
            for g in range(G):
                xx = xtp.tile([128, T], BF16, tag=f"xt{g}", name=f"xt{g}")
                xt_sb.append(xx)
            for h in range(NH):
                for g in range(G):
                    nc.sync.dma_start(
                        out=xt_sb[g][:, h * HALF:(h + 1) * HALF],
                        in_=xt_d[g * 128:(g + 1) * 128, h * HALF:(h + 1) * HALF])

            # layer-1 scan outputs (bf16), resident per (g, half)
            g1_sb = [[None] * NH for _ in range(G)]
            # persistent stats accumulators (PSUM)
            s13_ps = stats_ps.tile([72, TT], F32, tag="s13")
            s2_ps = stats_ps.tile([NT, TT], F32, tag="s2")
            stats_first = [True]
            stats_count = [0]
            N_STATS = G * NT         # stats matmul pairs = 32

            def sig_phase(layer, half, gate_r):
                """all zf/zi matmuls + sigmoids + den for one (layer, half).
                gate_r: previous phase's reciprocal tile (or None) used to
                gate this phase's biases (forces ScalarE phase ordering)."""
                if gate_r is None:
                    bsrc = lambda gate, g: bias_sb[:, layer, 4 * gate + g:4 * gate + g + 1]
                else:
                    bg = wk2.tile([128, 8], F32, tag="bgate")
                    nc.vector.tensor_tensor(
                        bg[:], bias_sb[:, layer], gate_r[:, 0:8], OP.min)
                    bsrc = lambda gate, g: bg[:, 4 * gate + g:4 * gate + g + 1]
                rhs = (xt_sb if layer == 0 else None)
                f_t, den_t = [], []
                for g in range(G):
                    f_t.append(gp.tile([128, HALF], BF16, tag=f"f{g}", name=f"f{g}"))
                    den_t.append(gp.tile([128, HALF], BF16, tag=f"den{g}", name=f"den{g}"))
                for ti in range(TPH):
                    t = half * TPH + ti
                    for g in range(G):
                        pf = gates_ps.tile([128, TT], F32, tag="pf")
                        pi = gates_ps.tile([128, TT], F32, tag="pi")
                        for gate, ps in ((0, pf), (1, pi)):
                            w = wt_sb[3 * layer + gate]
                            for k in range(K):
                                if layer == 0:
                                    r = xt_sb[k][:, t * TT:(t + 1) * TT]
                                else:
                                    r = g1_sb[k][half][:, ti * TT:(ti + 1) * TT]
                                nc.tensor.matmul(
                                    ps[:], w[:, k, g * 128:(g + 1) * 128], r,
                                    start=(k == 0), stop=(k == K - 1))
                        fs = f_t[g][:, ti * TT:(ti + 1) * TT]
                        nc.scalar.activation(fs, pf[:], AF.Sigmoid, bias=bsrc(0, g))
                        i_sb = work.tile([128, TT], BF16, tag="i")
                        nc.scalar.activation(i_sb[:], pi[:], AF.Sigmoid, bias=bsrc(1, g))
                        nc.vector.tensor_add(
                            den_t[g][:, ti * TT:(ti + 1) * TT], fs, i_sb[:])
                return f_t, den_t

            def rec_phase(layer, half, f_t, den_t, g2_carry):
                """reciprocal + a + u' + scan for one (layer, half).
                Returns (r_t3, gout list) for gating / the next stage."""
                r_t, a_t, up_t = [], [], []
                for g in range(G):
                    r = gp.tile([128, HALF], BF16, tag=f"r{g}", name=f"r{g}")
                    _act_direct(nc, r[:], den_t[g][:], AF.Reciprocal)
                    r_t.append(r)
                for g in range(G):
                    a = gp.tile([128, HALF], BF16, tag=f"den{g}", name=f"a{g}")
                    nc.vector.tensor_mul(a[:], f_t[g][:], r_t[g][:])
                    a_t.append(a)
                    up_t.append(gp.tile([128, HALF], BF16, tag=f"f{g}", name=f"up{g}"))
                for ti in range(TPH):
                    t = half * TPH + ti
                    for g in range(G):
                        ph = gates_ps.tile([128, TT], F32, tag="ph")
                        w = wt_sb[3 * layer + 2]
                        for k in range(K):
                            if layer == 0:
                                r = xt_sb[k][:, t * TT:(t + 1) * TT]
                            else:
                                r = g1_sb[k][half][:, ti * TT:(ti + 1) * TT]
                            nc.tensor.matmul(
                                ph[:], w[:, k, g * 128:(g + 1) * 128], r,
                                start=(k == 0), stop=(k == K - 1))
                        nc.vector.scalar_tensor_tensor(
                            up_t[g][:, ti * TT:(ti + 1) * TT],
                            a_t[g][:, ti * TT:(ti + 1) * TT], 1.0, ph[:],
                            OP.subtract, OP.mult)
                gout = []
                for g in range(G):
                    if layer == 0:
                        go = gp.tile([128, HALF], BF16, tag=f"g1_{g}_{half}", name=f"g1_{g}_{half}")
                        init = (gi_sb[:, 0, g:g + 1] if half == 0
                                else g1_sb[g][0][:, HALF - 1:HALF])
                        g1_sb[g][half] = go
                    else:
                        go = gp.tile([128, HALF], BF16, tag=f"g2_{g}", name=f"g2_{g}")
                        init = (gi_sb[:, 1, g:g + 1] if half == 0
                                else g2_carry[g][:])
                    nc.vector.tensor_tensor_scan(
                        go[:], a_t[g][:], up_t[g][:], init, OP.mult, OP.subtract)
                    gout.append(go)
                carry = None
                if layer == 1 and half == 0:
                    carry = []
                    for g in range(G):
                        cr = wk2.tile([128, 1], BF16, tag=f"carry{g}", name=f"carry{g}")
                        nc.vector.tensor_copy(cr[:], gout[g][:, HALF - 1:HALF])
                        carry.append(cr)
                return r_t[G - 1], gout, carry

            def epilogue(half, g2_t):
                """res = g2 + x^T, squares, stats matmuls for one half."""
                for g in range(G):
                    res = wk2.tile([128, HALF], BF16, tag="res")
                    nc.vector.tensor_add(
                        res[:], g2_t[g][:],
                        xt_sb[g][:, half * HALF:(half + 1) * HALF])
                    sq = wk2.tile([128, HALF], BF16, tag="sq")
                    nc.scalar.activation(sq[:], res[:], AF.Square)
                    for ti in range(TPH):
                        t = half * TPH + ti
                        rs = res[:, ti * TT:(ti + 1) * TT]
                        sqs = sq[:, ti * TT:(ti + 1) * TT]
                        first = stats_first[0]
                        stats_first[0] = False
                        stats_count[0] += 1
                        last = stats_count[0] == N_STATS
                        nc.tensor.matmul(
                            s13_ps[:], slt_sb[:, g, t, 0:72], rs,
                            start=first, stop=last, skip_group_check=True)
                        nc.tensor.matmul(
                            s2_ps[:], slt_sb[:, g, t, 0:8], sqs,
                            start=first, stop=last, skip_group_check=True)

            # ---- pipeline ----
            f_t, den_t = sig_phase(0, 0, None)
            r3, _, _ = rec_phase(0, 0, f_t, den_t, None)
            f_t, den_t = sig_phase(0, 1, r3)
            r3, _, _ = rec_phase(0, 1, f_t, den_t, None)
            f_t, den_t = sig_phase(1, 0, r3)
            r3, g2_t, carry = rec_phase(1, 0, f_t, den_t, None)
            epilogue(0, g2_t)
            f_t, den_t = sig_phase(1, 1, r3)
            _, g2_t, _ = rec_phase(1, 1, f_t, den_t, carry)
            epilogue(1, g2_t)

            # ---- final LN + projection math on [8, 512] ----
            # y = -( (s1_0*A - s3_0) + Kc ) * rsqrt(v + eps') + c0
            # v  = (s2_0 + 2*sc)/D - ((s1_0 + C1)/D)^2
            sc_sb = fin.tile([NT, TT], F32, tag="fin")
            nc.scalar.activation(sc_sb[:], s13_ps[64:64 + NT, :], AF.Copy)
            s3_sb = fin.tile([NT, TT], F32, tag="fin")
            nc.scalar.activation(s3_sb[:], s13_ps[32:32 + NT, :], AF.Copy)
            s2c = fin.tile([NT, TT], F32, tag="fin")
            nc.vector.scalar_tensor_tensor(
                s2c[:], sc_sb[:], 2.0, s2_ps[:], OP.mult, OP.add)
            mu2 = fin.tile([NT, TT], F32, tag="fin")
            nc.scalar.activation(mu2[:], s13_ps[0:NT, :], AF.Square,
                                 bias=epi_sb[:, 3:4], scale=1.0 / D)
            v = fin.tile([NT, TT], F32, tag="fin")
            nc.vector.scalar_tensor_tensor(
                v[:], s2c[:], 1.0 / D, mu2[:], OP.mult, OP.subtract)
            rv = fin.tile([NT, TT], F32, tag="fin")
            _act_direct(nc, rv[:], v[:], AF.Rsqrt, bias=epi_sb[:, 2:3])
            q = fin.tile([NT, TT], F32, tag="fin")
            nc.vector.scalar_tensor_tensor(
                q[:], s13_ps[0:NT, :], epi_sb[:, 1:2], s3_sb[:],
                OP.mult, OP.subtract)
            z = fin.tile([NT, TT], F32, tag="fin")
            nc.vector.scalar_tensor_tensor(
                z[:], q[:], epi_sb[:, 4:5], rv[:], OP.add, OP.mult)
            o_sb = fin.tile([NT, TT], F32, tag="fin")
            nc.scalar.activation(o_sb[:], z[:], AF.Identity,
                                 bias=epi_sb[:, 0:1], scale=-1.0)
            nc.sync.dma_start(out=out_d[:], in_=o_sb[:])

    _split_excess_waits(nc)
    return nc


_NC_CACHE = None


def _get_nc():
    global _NC_CACHE
    if _NC_CACHE is None:
        _NC_CACHE = _build_nc()
    return _NC_CACHE


def _host_prep(inputs):
    x = np.asarray(inputs["x"], dtype=np.float32)
    Ws = [np.asarray(inputs[n], np.float32) for n in
          ("Wf0", "Wi0", "Wh0", "Wf1", "Wi1", "Wh1")]
    bs = [np.asarray(inputs[n], np.float32) for n in
          ("bf0", "bi0", "bh0", "bf1", "bi1", "bh1")]
    bf0, bi0, bh0, bf1, bi1, bh1 = bs
    Wf1, Wi1, Wh1 = Ws[3], Ws[4], Ws[5]
    # h-bias elimination: layer-2 gate biases absorb Wx1 @ bh0
    bf1e = bf1 + Wf1 @ bh0
    bi1e = bi1 + Wi1 @ bh0
    bh1e = bh1 + Wh1 @ bh0

    wt_all = np.ascontiguousarray(
        np.stack([w.T for w in Ws])).astype(ml_dtypes.bfloat16)  # [6, din, dout]

    bias = np.zeros((128, 2, 8), np.float32)
    gi = np.zeros((128, 2, G), np.float32)
    for g in range(G):
        sl = slice(g * 128, (g + 1) * 128)
        bias[:, 0, 0 * 4 + g] = bf0[sl]
        bias[:, 0, 1 * 4 + g] = bi0[sl]
        bias[:, 1, 0 * 4 + g] = bf1e[sl]
        bias[:, 1, 1 * 4 + g] = bi1e[sl]
        gi[:, 0, g] = -bh0[sl]
        gi[:, 1, g] = -bh1e[sl]
    # min-gate trick requires |bias| < 0.5 <= r = 1/(f+i)
    assert np.abs(bias).max() < 0.45, "bias magnitude breaks min-gate trick"

    w_out = np.asarray(inputs["W_out"], np.float32).reshape(D)
    ln_g = np.asarray(inputs["ln_g"], np.float32)
    ln_b = np.asarray(inputs["ln_b"], np.float32)
    b_out = float(np.asarray(inputs["b_out"], np.float32).reshape(()))
    wg = w_out * ln_g
    c = bh1e                         # constant channel shift of res
    c0 = float(w_out @ ln_b) + b_out
    swg = float(wg.sum())
    C1 = float(c.sum())
    C2 = float((c * c).sum())
    C3 = float((wg * c).sum())
    A = swg / D
    Kc = C1 * A - C3
    epsP = LN_EPS + C2 / D
    C1D = C1 / D

    slt = np.zeros((G, NT, 128, 72), np.float32)
    for g in range(G):
        sl = slice(g * 128, (g + 1) * 128)
        for t in range(NT):
            slt[g, t, :, t] = 1.0
            slt[g, t, :, 32 + t] = wg[sl]
            slt[g, t, :, 64 + t] = c[sl]
    slt = slt.astype(ml_dtypes.bfloat16)

    epi = np.zeros((NT, 8), np.float32)
    epi[:, 0] = c0
    epi[:, 1] = A
    epi[:, 2] = epsP
    epi[:, 3] = C1D
    epi[:, 4] = Kc

    xt_b = np.ascontiguousarray(
        x.transpose(0, 2, 1)).astype(ml_dtypes.bfloat16)   # [B, D, T]
    return xt_b, wt_all, bias, gi, slt, epi


def _in_maps(inputs):
    xt_b, wt_all, bias, gi, slt, epi = _host_prep(inputs)
    return [
        {
            "xt": xt_b[b],
            "wt": wt_all, "bias": bias, "gi": gi, "slt": slt, "epi": epi,
        }
        for b in range(B)
    ]


def kernel(**inputs):
    nc = _get_nc()
    res = run_bass_kernel_spmd(nc, _in_maps(inputs), list(range(B)))
    out = np.stack([res.results[b]["out"].reshape(T, OUT) for b in range(B)])
    return out.astype(np.float32)


def kernel_traced(**inputs):
    """same as kernel() but returns (output, BassKernelResults) with timing"""
    nc = _get_nc()
    res = run_bass_kernel_spmd(nc, _in_maps(inputs), list(range(B)), trace=True)
    out = np.stack([res.results[b]["out"].reshape(T, OUT) for b in range(B)])
    return out.astype(np.float32), res


# revision 9
# speedup vs baseline: 1.2209x; 1.1495x over previous
"""Trainium2 Bass kernel for nn_DecoderMinLSTMGNN.

Model (per sample): two MinLSTM layers (D=512) over T=4096 steps, residual,
LayerNorm, projection D->1.  B=8 samples are data-parallel across the 8
NeuronCores (one sample per core).

Per-core layout is channels-major: x^T [D, T], bf16.  The time-dim linear
recurrence h_t = a_t*h_{t-1} + (1-a_t)*htilde_t maps onto the VectorE
TensorTensorScan instruction (scan along the free dim).

Key optimizations over the naive structure:
- h-gate bias elimination: substitute g = h - bh.  The recurrence becomes
  bias-free (g_t = a_t g_{t-1} + (1-a_t) zh_t with zh = W_h x, init -bh);
  the bias reappears as a constant shift that folds into the next layer's
  f/i gate biases (bf1_eff = bf1 + Wf1 @ bh0) and into the LN/projection
  stats via an extra lhsT column + host-side constants.  Removes 64
  bias matmuls.
- ScalarE act-table phase batching: sigmoids and reciprocals use different
  act-table sets (~1.3us per table load; the naive interleaving costs ~97
  loads = 124us).  Work is organized in (layer, half-of-T) phases: all 32
  sigmoids of a phase, then all reciprocals.  A "min-gate" artificial
  dependency (phase biases are routed through tensor_tensor(min) against
  the previous phase's reciprocal output, exact because |bias|<0.45 and
  r=1/(f+i)>=0.5) forces the scheduler to keep phases contiguous:
  9 table loads total.
- bf16 operands everywhere (matmuls, DVE tensor-tensor 2x mode, half the
  SBUF/DMA footprint).  PSUM accumulation and scan state stay fp32.
- Wide [128, 2048] instructions for reciprocal/square/a/den/res/scan to
  amortize per-instruction overhead.
"""

import numpy as np
import ml_dtypes

import concourse.bass as bass
import concourse.mybir as mybir
import concourse.tile as tile
from concourse.bass_utils import run_bass_kernel_spmd

F32 = mybir.dt.float32
BF16 = mybir.dt.bfloat16
AF = mybir.ActivationFunctionType
OP = mybir.AluOpType

B, T, D = 8, 4096, 512
OUT = 1
LN_EPS = 1e-5
TT = 512                 # time-tile size
NT = T // TT             # 8 time tiles
G = D // 128             # 4 channel groups
K = D // 128             # 4 contraction chunks
TPH = 4                  # time tiles per phase (half)
HALF = TPH * TT          # 2048
NH = NT // TPH           # 2 halves

MAX_WAITS = 1


def _split_excess_waits(nc):
    """walrus in this container rejects >1 semaphore wait per instruction
    ("Too many sync wait commands"); move excess waits onto NoOps."""
    for fn in nc.m.functions:
        for bb in fn.blocks:
            new_list = []
            changed = False
            for inst in bb.instructions:
                si = inst.sync_info
                waits = list(si.on_wait) if si is not None and si.on_wait else []
                if len(waits) > MAX_WAITS:
                    changed = True
                    overflow = waits[:-MAX_WAITS]
                    si.on_wait = waits[-MAX_WAITS:]
                    for j in range(0, len(overflow), MAX_WAITS):
                        new_list.append(mybir.InstNoOp(
                            name=f"{inst.name}-waitsplit-{j}",
                            engine=inst.engine,
                            ins=[], outs=[],
                            sync_info=mybir.SyncInfo(
                                on_wait=overflow[j:j + MAX_WAITS], on_update=[]),
                        ))
                new_list.append(inst)
            if changed:
                bb.instructions[:] = new_list
    return nc


def _act_direct(nc, out, in_, func, bias=0.0, scale=1.0):
    """emit InstActivation directly (bass blocks Reciprocal/Rsqrt)."""
    ins = [nc.scalar.lower_ap(in_)]
    for v in (bias, scale, 0.0):
        if isinstance(v, (int, float)):
            ins.append(mybir.ImmediateValue(dtype=mybir.dt.float32, value=float(v)))
        else:
            ins.append(nc.scalar.lower_ap(v))
    return nc.scalar.add_instruction(
        mybir.InstActivation(
            name=nc.get_next_instruction_name(),
            func=func, ins=ins, outs=[nc.scalar.lower_ap(out)]))


def _build_nc():
    nc = bass.Bass()

    xt_d = nc.dram_tensor("xt", [D, T], BF16, kind="ExternalInput")
    wt_d = nc.dram_tensor("wt", [6, D, D], BF16, kind="ExternalInput")
    # f/i gate biases (layer-2 ones pre-corrected): [128, layer, gate*4+g]
    bias_d = nc.dram_tensor("bias", [128, 2, 8], F32, kind="ExternalInput")
    # scan initial state columns (-bh_eff): [128, layer, g]
    gi_d = nc.dram_tensor("gi", [128, 2, G], F32, kind="ExternalInput")
    # stats lhsT per (g,t): col t = 1 (s1), col 32+t = wg, col 64+t = c
    slt_d = nc.dram_tensor("slt", [G, NT, 128, 72], BF16, kind="ExternalInput")
    epi_d = nc.dram_tensor("epi", [NT, 8], F32, kind="ExternalInput")
    out_d = nc.dram_tensor("out", [NT, TT], F32, kind="ExternalOutput")

    with tile.TileContext(nc) as tc:
        with (
            tc.tile_pool(name="const", bufs=1) as const,
            tc.tile_pool(name="xtp", bufs=1) as xtp,
            tc.tile_pool(name="gp", bufs=1) as gp,        # wide per-g phase bufs
            tc.tile_pool(name="work", bufs=2) as work,    # i tiles
            tc.tile_pool(name="wk2", bufs=2) as wk2,      # res/sq/bgate/carry
            tc.tile_pool(name="fin", bufs=4) as fin,
            tc.tile_pool(name="gates_ps", bufs=2, space="PSUM") as gates_ps,
            tc.tile_pool(name="stats_ps", bufs=1, space="PSUM") as stats_ps,
        ):
            # ---- constants + x, DMA-ordered so phase (0,0) starts ASAP ----
            wt_sb = [None] * 6
            def _load_w(idx):
                w = const.tile([128, K, D], BF16, tag=f"wt{idx}", name=f"wt{idx}")
                nc.sync.dma_start(
                    out=w[:], in_=wt_d[idx].rearrange("(k p) d -> p k d", p=128))
                wt_sb[idx] = w
            bias_sb = const.tile([128, 2, 8], F32)
            nc.sync.dma_start(out=bias_sb[:], in_=bias_d[:])
            _load_w(0)
            _load_w(1)
            xt_sb = []
            for g in range(G):
                xx = xtp.tile([128, T], BF16, tag=f"xt{g}", name=f"xt{g}")
                xt_sb.append(xx)
            QU = HALF // 2
            for q in range(2):
                for g in range(G):
                    nc.sync.dma_start(
                        out=xt_sb[g][:, q * QU:(q + 1) * QU],
                        in_=xt_d[g * 128:(g + 1) * 128, q * QU:(q + 1) * QU])
            _load_w(2)
            gi_sb = const.tile([128, 2, G], F32)
            nc.sync.dma_start(out=gi_sb[:], in_=gi_d[:])
            for g in range(G):
                nc.sync.dma_start(
                    out=xt_sb[g][:, HALF:T],
                    in_=xt_d[g * 128:(g + 1) * 128, HALF:T])
            _load_w(3)
            _load_w(4)
            _load_w(5)
            slt_sb = const.tile([128, G, NT, 72], BF16)
            nc.sync.dma_start(
                out=slt_sb[:], in_=slt_d.rearrange("g t p c -> p g t c"))
            epi_sb = const.tile([NT, 8], F32)
            nc.sync.dma_start(out=epi_sb[:], in_=epi_d[:])

            # layer-1 scan outputs (bf16), resident per (g, half)
            g1_sb = [[None] * NH for _ in range(G)]
            # persistent stats accumulators (PSUM)
            s13_ps = stats_ps.tile([72, TT], F32, tag="s13")
            s2_ps = stats_ps.tile([NT, TT], F32, tag="s2")
            stats_first = [True]
            stats_count = [0]
            N_STATS = G * NT         # stats matmul pairs = 32

            def sig_phase(layer, half, gate_r):
                """all zf/zi matmuls + sigmoids + den for one (layer, half).
                gate_r: previous phase's reciprocal tile (or None) used to
                gate this phase's biases (forces ScalarE phase ordering)."""
                if gate_r is None:
                    bsrc = lambda gate, g: bias_sb[:, layer, 4 * gate + g:4 * gate + g + 1]
                else:
                    bg = wk2.tile([128, 8], F32, tag="bgate")
                    nc.vector.tensor_tensor(
                        bg[:], bias_sb[:, layer], gate_r[0][:, 0:8], OP.min)
                    for rr in gate_r[1:]:
                        bg2 = wk2.tile([128, 8], F32, tag="bgate", name="bg2")
                        nc.vector.tensor_tensor(bg2[:], bg[:], rr[:, 0:8], OP.min)
                        bg = bg2
                    bsrc = lambda gate, g: bg[:, 4 * gate + g:4 * gate + g + 1]
                rhs = (xt_sb if layer == 0 else None)
                f_t, den_t = [], []
                for g in range(G):
                    f_t.append(gp.tile([128, HALF], BF16, tag=f"fr{g}", bufs=2, name=f"f{g}"))
                    den_t.append(gp.tile([128, HALF], BF16, tag=f"da{g}", bufs=2, name=f"den{g}"))
                for ti in range(TPH):
                    t = half * TPH + ti
                    for g in range(G):
                        pf = gates_ps.tile([128, TT], F32, tag="pf")
                        pi = gates_ps.tile([128, TT], F32, tag="pi")
                        for gate, ps in ((0, pf), (1, pi)):
                            w = wt_sb[3 * layer + gate]
                            for k in range(K):
                                if layer == 0:
                                    r = xt_sb[k][:, t * TT:(t + 1) * TT]
                                else:
                                    r = g1_sb[k][half][:, ti * TT:(ti + 1) * TT]
                                nc.tensor.matmul(
                                    ps[:], w[:, k, g * 128:(g + 1) * 128], r,
                                    start=(k == 0), stop=(k == K - 1))
                        fs = f_t[g][:, ti * TT:(ti + 1) * TT]
                        nc.scalar.activation(fs, pf[:], AF.Sigmoid, bias=bsrc(0, g))
                        i_sb = work.tile([128, TT], BF16, tag="i")
                        nc.scalar.activation(i_sb[:], pi[:], AF.Sigmoid, bias=bsrc(1, g))
                        nc.vector.tensor_add(
                            den_t[g][:, ti * TT:(ti + 1) * TT], fs, i_sb[:])
                return f_t, den_t

            def rec_phase(layer, half, f_t, den_t, g2_carry):
                """reciprocal + a + u' for one (layer, half); scans are
                emitted later (emit_scans closure) so the next sig phase's
                den adds aren't queued behind them on DVE."""
                r_t, a_t, up_t = [], [], []
                for g in range(G):
                    r = gp.tile([128, HALF], BF16, tag=f"fr{g}", bufs=2, name=f"r{g}")
                    _act_direct(nc, r[:], den_t[g][:], AF.Reciprocal)
                    r_t.append(r)
                for g in range(G):
                    a = gp.tile([128, HALF], BF16, tag=f"da{g}", bufs=2, name=f"a{g}")
                    nc.vector.tensor_mul(a[:], f_t[g][:], r_t[g][:])
                    a_t.append(a)
                    up_t.append(gp.tile([128, HALF], BF16, tag=f"up{g}", bufs=1, name=f"up{g}"))
                for ti in range(TPH):
                    t = half * TPH + ti
                    for g in range(G):
                        ph = gates_ps.tile([128, TT], F32, tag="ph")
                        w = wt_sb[3 * layer + 2]
                        for k in range(K):
                            if layer == 0:
                                r = xt_sb[k][:, t * TT:(t + 1) * TT]
                            else:
                                r = g1_sb[k][half][:, ti * TT:(ti + 1) * TT]
                            nc.tensor.matmul(
                                ph[:], w[:, k, g * 128:(g + 1) * 128], r,
                                start=(k == 0), stop=(k == K - 1))
                        nc.vector.scalar_tensor_tensor(
                            up_t[g][:, ti * TT:(ti + 1) * TT],
                            a_t[g][:, ti * TT:(ti + 1) * TT], 1.0, ph[:],
                            OP.subtract, OP.mult)
                def emit_scans():
                    gout = []
                    for g in range(G):
                        if layer == 0:
                            go = gp.tile([128, HALF], BF16, tag=f"g1_{g}_{half}", name=f"g1_{g}_{half}")
                            init = (gi_sb[:, 0, g:g + 1] if half == 0
                                    else g1_sb[g][0][:, HALF - 1:HALF])
                            g1_sb[g][half] = go
                        else:
                            go = gp.tile([128, HALF], BF16, tag=f"g2_{g}", name=f"g2_{g}")
                            init = (gi_sb[:, 1, g:g + 1] if half == 0
                                    else g2_carry[g][:])
                        nc.vector.tensor_tensor_scan(
                            go[:], a_t[g][:], up_t[g][:], init, OP.mult, OP.subtract)
                        gout.append(go)
                    carry = None
                    if layer == 1 and half == 0:
                        carry = []
                        for g in range(G):
                            cr = wk2.tile([128, 1], BF16, tag=f"carry{g}", name=f"carry{g}")
                            nc.vector.tensor_copy(cr[:], gout[g][:, HALF - 1:HALF])
                            carry.append(cr)
                    return gout, carry
                return r_t, emit_scans

            def epilogue(half, g2_t):
                """res = g2 + x^T, squares, stats matmuls for one half."""
                for g in range(G):
                    res = wk2.tile([128, HALF], BF16, tag="res", bufs=1)
                    nc.vector.tensor_add(
                        res[:], g2_t[g][:],
                        xt_sb[g][:, half * HALF:(half + 1) * HALF])
                    sq = wk2.tile([128, HALF], BF16, tag="sq", bufs=1)
                    nc.scalar.activation(sq[:], res[:], AF.Square)
                    for ti in range(TPH):
                        t = half * TPH + ti
                        rs = res[:, ti * TT:(ti + 1) * TT]
                        sqs = sq[:, ti * TT:(ti + 1) * TT]
                        first = stats_first[0]
                        stats_first[0] = False
                        stats_count[0] += 1
                        last = stats_count[0] == N_STATS
                        nc.tensor.matmul(
                            s13_ps[:], slt_sb[:, g, t, 0:72], rs,
                            start=first, stop=last, skip_group_check=True)
                        nc.tensor.matmul(
                            s2_ps[:], slt_sb[:, g, t, 0:8], sqs,
                            start=first, stop=last, skip_group_check=True)

            # ---- pipeline (scans deferred past the next sig phase) ----
            f_t, den_t = sig_phase(0, 0, None)
            r_a, sc_a = rec_phase(0, 0, f_t, den_t, None)
            f_t, den_t = sig_phase(0, 1, r_a)
            sc_a()
            r_b, sc_b = rec_phase(0, 1, f_t, den_t, None)
            f_t, den_t = sig_phase(1, 0, r_b)
            sc_b()
            r_c, sc_c = rec_phase(1, 0, f_t, den_t, None)
            f_t, den_t = sig_phase(1, 1, r_c)
            g2_t, carry = sc_c()
            epilogue(0, g2_t)
            r_d, sc_d = rec_phase(1, 1, f_t, den_t, carry)
            g2_t, _ = sc_d()
            epilogue(1, g2_t)

            # ---- final LN + projection math on [8, 512] ----
            # y = -( (s1_0*A - s3_0) + Kc ) * rsqrt(v + eps') + c0
            # v  = (s2_0 + 2*sc)/D - ((s1_0 + C1)/D)^2
            sc_sb = fin.tile([NT, TT], F32, tag="fin")
            nc.scalar.activation(sc_sb[:], s13_ps[64:64 + NT, :], AF.Copy)
            s3_sb = fin.tile([NT, TT], F32, tag="fin")
            nc.scalar.activation(s3_sb[:], s13_ps[32:32 + NT, :], AF.Copy)
            s2c = fin.tile([NT, TT], F32, tag="fin")
            nc.vector.scalar_tensor_tensor(
                s2c[:], sc_sb[:], 2.0, s2_ps[:], OP.mult, OP.add)
            mu2 = fin.tile([NT, TT], F32, tag="fin")
            nc.scalar.activation(mu2[:], s13_ps[0:NT, :], AF.Square,
                                 bias=epi_sb[:, 3:4], scale=1.0 / D)
            v = fin.tile([NT, TT], F32, tag="fin")
            nc.vector.scalar_tensor_tensor(
                v[:], s2c[:], 1.0 / D, mu2[:], OP.mult, OP.subtract)
            rv = fin.tile([NT, TT], F32, tag="fin")
            _act_direct(nc, rv[:], v[:], AF.Rsqrt, bias=epi_sb[:, 2:3])
            q = fin.tile([NT, TT], F32, tag="fin")
            nc.vector.scalar_tensor_tensor(
                q[:], s13_ps[0:NT, :], epi_sb[:, 1:2], s3_sb[:],
                OP.mult, OP.subtract)
            z = fin.tile([NT, TT], F32, tag="fin")
            nc.vector.scalar_tensor_tensor(
                z[:], q[:], epi_sb[:, 4:5], rv[:], OP.add, OP.mult)
            o_sb = fin.tile([NT, TT], F32, tag="fin")
            nc.scalar.activation(o_sb[:], z[:], AF.Identity,
                                 bias=epi_sb[:, 0:1], scale=-1.0)
            nc.sync.dma_start(out=out_d[:], in_=o_sb[:])

    _split_excess_waits(nc)
    return nc


_NC_CACHE = None


def _get_nc():
    global _NC_CACHE
    if _NC_CACHE is None:
        _NC_CACHE = _build_nc()
    return _NC_CACHE


def _host_prep(inputs):
    x = np.asarray(inputs["x"], dtype=np.float32)
    Ws = [np.asarray(inputs[n], np.float32) for n in
          ("Wf0", "Wi0", "Wh0", "Wf1", "Wi1", "Wh1")]
    bs = [np.asarray(inputs[n], np.float32) for n in
          ("bf0", "bi0", "bh0", "bf1", "bi1", "bh1")]
    bf0, bi0, bh0, bf1, bi1, bh1 = bs
    Wf1, Wi1, Wh1 = Ws[3], Ws[4], Ws[5]
    # h-bias elimination: layer-2 gate biases absorb Wx1 @ bh0
    bf1e = bf1 + Wf1 @ bh0
    bi1e = bi1 + Wi1 @ bh0
    bh1e = bh1 + Wh1 @ bh0

    wt_all = np.ascontiguousarray(
        np.stack([w.T for w in Ws])).astype(ml_dtypes.bfloat16)  # [6, din, dout]

    bias = np.zeros((128, 2, 8), np.float32)
    gi = np.zeros((128, 2, G), np.float32)
    for g in range(G):
        sl = slice(g * 128, (g + 1) * 128)
        bias[:, 0, 0 * 4 + g] = bf0[sl]
        bias[:, 0, 1 * 4 + g] = bi0[sl]
        bias[:, 1, 0 * 4 + g] = bf1e[sl]
        bias[:, 1, 1 * 4 + g] = bi1e[sl]
        gi[:, 0, g] = -bh0[sl]
        gi[:, 1, g] = -bh1e[sl]
    # min-gate trick requires |bias| < 0.5 <= r = 1/(f+i)
    assert np.abs(bias).max() < 0.45, "bias magnitude breaks min-gate trick"

    w_out = np.asarray(inputs["W_out"], np.float32).reshape(D)
    ln_g = np.asarray(inputs["ln_g"], np.float32)
    ln_b = np.asarray(inputs["ln_b"], np.float32)
    b_out = float(np.asarray(inputs["b_out"], np.float32).reshape(()))
    wg = w_out * ln_g
    c = bh1e                         # constant channel shift of res
    c0 = float(w_out @ ln_b) + b_out
    swg = float(wg.sum())
    C1 = float(c.sum())
    C2 = float((c * c).sum())
    C3 = float((wg * c).sum())
    A = swg / D
    Kc = C1 * A - C3
    epsP = LN_EPS + C2 / D
    C1D = C1 / D

    slt = np.zeros((G, NT, 128, 72), np.float32)
    for g in range(G):
        sl = slice(g * 128, (g + 1) * 128)
        for t in range(NT):
            slt[g, t, :, t] = 1.0
            slt[g, t, :, 32 + t] = wg[sl]
            slt[g, t, :, 64 + t] = c[sl]
    slt = slt.astype(ml_dtypes.bfloat16)

    epi = np.zeros((NT, 8), np.float32)
    epi[:, 0] = c0
    epi[:, 1] = A
    epi[:, 2] = epsP
    epi[:, 3] = C1D
    epi[:, 4] = Kc

    xt_b = np.ascontiguousarray(
        x.transpose(0, 2, 1)).astype(ml_dtypes.bfloat16)   # [B, D, T]
    return xt_b, wt_all, bias, gi, slt, epi


def _in_maps(inputs):
    xt_b, wt_all, bias, gi, slt, epi = _host_prep(inputs)
    return [
        {
            "xt": xt_b[b],
            "wt": wt_all, "bias": bias, "gi": gi, "slt": slt, "epi": epi,
        }
        for b in range(B)
    ]


def kernel(**inputs):
    nc = _get_nc()
    res = run_bass_kernel_spmd(nc, _in_maps(inputs), list(range(B)))
    out = np.stack([res.results[b]["out"].reshape(T, OUT) for b in range(B)])
    return out.astype(np.float32)


def kernel_traced(**inputs):
    """same as kernel() but returns (output, BassKernelResults) with timing"""
    nc = _get_nc()
    res = run_bass_kernel_spmd(nc, _in_maps(inputs), list(range(B)), trace=True)
    out = np.stack([res.results[b]["out"].reshape(T, OUT) for b in range(B)])
    return out.astype(np.float32), res


# revision 10
# speedup vs baseline: 1.2752x; 1.0444x over previous
"""Trainium2 Bass kernel for nn_DecoderMinLSTMGNN.

Model (per sample): two MinLSTM layers (D=512) over T=4096 steps, residual,
LayerNorm, projection D->1.  B=8 samples are data-parallel across the 8
NeuronCores (one sample per core).

Per-core layout is channels-major: x^T [D, T], bf16.  The time-dim linear
recurrence h_t = a_t*h_{t-1} + (1-a_t)*htilde_t maps onto the VectorE
TensorTensorScan instruction (scan along the free dim).

Key optimizations over the naive structure:
- h-gate bias elimination: substitute g = h - bh.  The recurrence becomes
  bias-free (g_t = a_t g_{t-1} + (1-a_t) zh_t with zh = W_h x, init -bh);
  the bias reappears as a constant shift that folds into the next layer's
  f/i gate biases (bf1_eff = bf1 + Wf1 @ bh0) and into the LN/projection
  stats via an extra lhsT column + host-side constants.  Removes 64
  bias matmuls.
- ScalarE act-table phase batching: sigmoids and reciprocals use different
  act-table sets (~1.3us per table load; the naive interleaving costs ~97
  loads = 124us).  Work is organized in (layer, half-of-T) phases: all 32
  sigmoids of a phase, then all reciprocals.  A "min-gate" artificial
  dependency (phase biases are routed through tensor_tensor(min) against
  the previous phase's reciprocal output, exact because |bias|<0.45 and
  r=1/(f+i)>=0.5) forces the scheduler to keep phases contiguous:
  9 table loads total.
- bf16 operands everywhere (matmuls, DVE tensor-tensor 2x mode, half the
  SBUF/DMA footprint).  PSUM accumulation and scan state stay fp32.
- Wide [128, 2048] instructions for reciprocal/square/a/den/res/scan to
  amortize per-instruction overhead.
"""

import numpy as np
import ml_dtypes

import concourse.bass as bass
import concourse.mybir as mybir
import concourse.tile as tile
from concourse.bass_utils import run_bass_kernel_spmd

F32 = mybir.dt.float32
BF16 = mybir.dt.bfloat16
AF = mybir.ActivationFunctionType
OP = mybir.AluOpType

B, T, D = 8, 4096, 512
OUT = 1
LN_EPS = 1e-5
TT = 512                 # time-tile size
NT = T // TT             # 8 time tiles
G = D // 128             # 4 channel groups
K = D // 128             # 4 contraction chunks
TPH = 4                  # time tiles per phase (half)
HALF = TPH * TT          # 2048
NH = NT // TPH           # 2 halves

MAX_WAITS = 1


def _split_excess_waits(nc):
    """walrus in this container rejects >1 semaphore wait per instruction
    ("Too many sync wait commands"); move excess waits onto NoOps."""
    for fn in nc.m.functions:
        for bb in fn.blocks:
            new_list = []
            changed = False
            for inst in bb.instructions:
                si = inst.sync_info
                waits = list(si.on_wait) if si is not None and si.on_wait else []
                if len(waits) > MAX_WAITS:
                    changed = True
                    overflow = waits[:-MAX_WAITS]
                    si.on_wait = waits[-MAX_WAITS:]
                    for j in range(0, len(overflow), MAX_WAITS):
                        new_list.append(mybir.InstNoOp(
                            name=f"{inst.name}-waitsplit-{j}",
                            engine=inst.engine,
                            ins=[], outs=[],
                            sync_info=mybir.SyncInfo(
                                on_wait=overflow[j:j + MAX_WAITS], on_update=[]),
                        ))
                new_list.append(inst)
            if changed:
                bb.instructions[:] = new_list
    return nc


def _act_direct(nc, out, in_, func, bias=0.0, scale=1.0):
    """emit InstActivation directly (bass blocks Reciprocal/Rsqrt)."""
    ins = [nc.scalar.lower_ap(in_)]
    for v in (bias, scale, 0.0):
        if isinstance(v, (int, float)):
            ins.append(mybir.ImmediateValue(dtype=mybir.dt.float32, value=float(v)))
        else:
            ins.append(nc.scalar.lower_ap(v))
    return nc.scalar.add_instruction(
        mybir.InstActivation(
            name=nc.get_next_instruction_name(),
            func=func, ins=ins, outs=[nc.scalar.lower_ap(out)]))


def _build_nc():
    nc = bass.Bass()

    xt_d = nc.dram_tensor("xt", [D, T], BF16, kind="ExternalInput")
    wt_d = nc.dram_tensor("wt", [6, D, D], BF16, kind="ExternalInput")
    # f/i gate biases (layer-2 ones pre-corrected): [128, layer, gate*4+g]
    bias_d = nc.dram_tensor("bias", [128, 2, 8], F32, kind="ExternalInput")
    # scan initial state columns (-bh_eff): [128, layer, g]
    gi_d = nc.dram_tensor("gi", [128, 2, G], F32, kind="ExternalInput")
    # stats lhsT per (g,t): col t = 1 (s1), col 32+t = wg, col 64+t = c
    slt_d = nc.dram_tensor("slt", [G, NT, 128, 72], BF16, kind="ExternalInput")
    epi_d = nc.dram_tensor("epi", [NT, 8], F32, kind="ExternalInput")
    out_d = nc.dram_tensor("out", [NT, TT], F32, kind="ExternalOutput")

    with tile.TileContext(nc) as tc:
        with (
            tc.tile_pool(name="const", bufs=1) as const,
            tc.tile_pool(name="xtp", bufs=1) as xtp,
            tc.tile_pool(name="gp", bufs=1) as gp,        # wide per-g phase bufs
            tc.tile_pool(name="work", bufs=2) as work,    # i tiles
            tc.tile_pool(name="wk2", bufs=2) as wk2,      # res/sq/bgate/carry
            tc.tile_pool(name="fin", bufs=4) as fin,
            tc.tile_pool(name="gates_ps", bufs=6, space="PSUM") as gates_ps,
            tc.tile_pool(name="stats_ps", bufs=1, space="PSUM") as stats_ps,
        ):
            # ---- constants + x, DMA-ordered so phase (0,0) starts ASAP ----
            wt_sb = [None] * 6
            def _load_w(idx):
                w = const.tile([128, K, D], BF16, tag=f"wt{idx}", name=f"wt{idx}")
                nc.sync.dma_start(
                    out=w[:], in_=wt_d[idx].rearrange("(k p) d -> p k d", p=128))
                wt_sb[idx] = w
            bias_sb = const.tile([128, 2, 8], F32)
            nc.sync.dma_start(out=bias_sb[:], in_=bias_d[:])
            _load_w(0)
            _load_w(1)
            xt_sb = []
            for g in range(G):
                xx = xtp.tile([128, T], BF16, tag=f"xt{g}", name=f"xt{g}")
                xt_sb.append(xx)
            QU = HALF // 2
            for q in range(2):
                for g in range(G):
                    nc.gpsimd.dma_start(
                        out=xt_sb[g][:, q * QU:(q + 1) * QU],
                        in_=xt_d[g * 128:(g + 1) * 128, q * QU:(q + 1) * QU])
            _load_w(2)
            gi_sb = const.tile([128, 2, G], F32)
            nc.sync.dma_start(out=gi_sb[:], in_=gi_d[:])
            for g in range(G):
                nc.gpsimd.dma_start(
                    out=xt_sb[g][:, HALF:T],
                    in_=xt_d[g * 128:(g + 1) * 128, HALF:T])
            _load_w(3)
            _load_w(4)
            _load_w(5)
            slt_sb = const.tile([128, G, NT, 72], BF16)
            nc.sync.dma_start(
                out=slt_sb[:], in_=slt_d.rearrange("g t p c -> p g t c"))
            epi_sb = const.tile([NT, 8], F32)
            nc.sync.dma_start(out=epi_sb[:], in_=epi_d[:])

            # layer-1 scan outputs (bf16), resident per (g, half)
            g1_sb = [[None] * NH for _ in range(G)]
            # persistent stats accumulators (PSUM)
            s13_ps = stats_ps.tile([72, TT], F32, tag="s13")
            s2_ps = stats_ps.tile([NT, TT], F32, tag="s2")
            stats_first = [True]
            stats_count = [0]
            N_STATS = G * NT         # stats matmul pairs = 32

            def sig_phase(layer, half, gate_r):
                """all zf/zi matmuls + sigmoids + den for one (layer, half).
                gate_r: previous phase's reciprocal tile (or None) used to
                gate this phase's biases (forces ScalarE phase ordering)."""
                if gate_r is None:
                    bsrc = lambda gate, g: bias_sb[:, layer, 4 * gate + g:4 * gate + g + 1]
                else:
                    bg = wk2.tile([128, 8], F32, tag="bgate")
                    nc.vector.tensor_tensor(
                        bg[:], bias_sb[:, layer], gate_r[0][:, 0:8], OP.min)
                    for rr in gate_r[1:]:
                        bg2 = wk2.tile([128, 8], F32, tag="bgate", name="bg2")
                        nc.vector.tensor_tensor(bg2[:], bg[:], rr[:, 0:8], OP.min)
                        bg = bg2
                    bsrc = lambda gate, g: bg[:, 4 * gate + g:4 * gate + g + 1]
                rhs = (xt_sb if layer == 0 else None)
                f_t, den_t = [], []
                for g in range(G):
                    f_t.append(gp.tile([128, HALF], BF16, tag=f"fr{g}", bufs=2, name=f"f{g}"))
                    den_t.append(gp.tile([128, HALF], BF16, tag=f"da{g}", bufs=2, name=f"den{g}"))
                for ti in range(TPH):
                    t = half * TPH + ti
                    for g in range(G):
                        pf = gates_ps.tile([128, TT], F32, tag="mm", name="pf")
                        pi = gates_ps.tile([128, TT], F32, tag="mm", name="pi")
                        for gate, ps in ((0, pf), (1, pi)):
                            w = wt_sb[3 * layer + gate]
                            for k in range(K):
                                if layer == 0:
                                    r = xt_sb[k][:, t * TT:(t + 1) * TT]
                                else:
                                    r = g1_sb[k][half][:, ti * TT:(ti + 1) * TT]
                                nc.tensor.matmul(
                                    ps[:], w[:, k, g * 128:(g + 1) * 128], r,
                                    start=(k == 0), stop=(k == K - 1))
                        fs = f_t[g][:, ti * TT:(ti + 1) * TT]
                        nc.scalar.activation(fs, pf[:], AF.Sigmoid, bias=bsrc(0, g))
                        i_sb = work.tile([128, TT], BF16, tag="i")
                        nc.scalar.activation(i_sb[:], pi[:], AF.Sigmoid, bias=bsrc(1, g))
                        nc.vector.tensor_add(
                            den_t[g][:, ti * TT:(ti + 1) * TT], fs, i_sb[:])
                        i_last = i_sb
                # gate all reciprocals on the last sigmoid of the phase:
                # max(den, i) == den exactly (den = f+i >= i), so this only
                # adds the dependency, keeping the act-table phases contiguous.
                for g in range(G):
                    nc.vector.tensor_tensor(
                        den_t[g][:, 0:1], den_t[g][:, 0:1], i_last[:, 0:1], OP.max)
                return f_t, den_t

            def rec_phase(layer, half, f_t, den_t, g2_carry):
                """reciprocal + a + u' for one (layer, half); scans are
                emitted later (emit_scans closure) so the next sig phase's
                den adds aren't queued behind them on DVE."""
                r_t, a_t, up_t = [], [], []
                for g in range(G):
                    r = gp.tile([128, HALF], BF16, tag=f"fr{g}", bufs=2, name=f"r{g}")
                    _act_direct(nc, r[:], den_t[g][:], AF.Reciprocal)
                    r_t.append(r)
                for g in range(G):
                    a = gp.tile([128, HALF], BF16, tag=f"da{g}", bufs=2, name=f"a{g}")
                    nc.vector.tensor_mul(a[:], f_t[g][:], r_t[g][:])
                    a_t.append(a)
                    up_t.append(gp.tile([128, HALF], BF16, tag=f"up{g}", bufs=1, name=f"up{g}"))
                for ti in range(TPH):
                    t = half * TPH + ti
                    for g in range(G):
                        ph = gates_ps.tile([128, TT], F32, tag="mm", name="ph")
                        w = wt_sb[3 * layer + 2]
                        for k in range(K):
                            if layer == 0:
                                r = xt_sb[k][:, t * TT:(t + 1) * TT]
                            else:
                                r = g1_sb[k][half][:, ti * TT:(ti + 1) * TT]
                            nc.tensor.matmul(
                                ph[:], w[:, k, g * 128:(g + 1) * 128], r,
                                start=(k == 0), stop=(k == K - 1))
                        nc.vector.scalar_tensor_tensor(
                            up_t[g][:, ti * TT:(ti + 1) * TT],
                            a_t[g][:, ti * TT:(ti + 1) * TT], 1.0, ph[:],
                            OP.subtract, OP.mult)
                def emit_scans(epi_g=None):
                    gout = []
                    carry = [] if (layer == 1 and half == 0) else None
                    for g in range(G):
                        if layer == 0:
                            go = gp.tile([128, HALF], BF16, tag=f"g1_{g}_{half}", name=f"g1_{g}_{half}")
                            init = (gi_sb[:, 0, g:g + 1] if half == 0
                                    else g1_sb[g][0][:, HALF - 1:HALF])
                            g1_sb[g][half] = go
                        else:
                            go = gp.tile([128, HALF], BF16, tag=f"g2_{g}", name=f"g2_{g}")
                            init = (gi_sb[:, 1, g:g + 1] if half == 0
                                    else g2_carry[g][:])
                        nc.vector.tensor_tensor_scan(
                            go[:], a_t[g][:], up_t[g][:], init, OP.mult, OP.subtract)
                        gout.append(go)
                        if carry is not None:
                            cr = wk2.tile([128, 1], BF16, tag=f"carry{g}", name=f"carry{g}")
                            nc.vector.tensor_copy(cr[:], go[:, HALF - 1:HALF])
                            carry.append(cr)
                        if epi_g is not None:
                            epi_g(g, go)
                    return gout, carry
                return r_t, emit_scans

            def epilogue(half):
                """per-g closure: res = g2 + x^T, square, stats matmuls."""
                def epi_g(g, g2):
                    res = wk2.tile([128, HALF], BF16, tag="res", bufs=1)
                    nc.vector.tensor_add(
                        res[:], g2[:],
                        xt_sb[g][:, half * HALF:(half + 1) * HALF])
                    sq = wk2.tile([128, HALF], BF16, tag="sq", bufs=1)
                    nc.scalar.activation(sq[:], res[:], AF.Square)
                    for ti in range(TPH):
                        t = half * TPH + ti
                        rs = res[:, ti * TT:(ti + 1) * TT]
                        sqs = sq[:, ti * TT:(ti + 1) * TT]
                        first = stats_first[0]
                        stats_first[0] = False
                        stats_count[0] += 1
                        last = stats_count[0] == N_STATS
                        nc.tensor.matmul(
                            s13_ps[:], slt_sb[:, g, t, 0:72], rs,
                            start=first, stop=last, skip_group_check=True)
                        nc.tensor.matmul(
                            s2_ps[:], slt_sb[:, g, t, 0:8], sqs,
                            start=first, stop=last, skip_group_check=True)
                return epi_g

            # ---- pipeline (scans deferred past the next sig phase) ----
            f_t, den_t = sig_phase(0, 0, None)
            r_a, sc_a = rec_phase(0, 0, f_t, den_t, None)
            f_t, den_t = sig_phase(0, 1, r_a)
            sc_a()
            r_b, sc_b = rec_phase(0, 1, f_t, den_t, None)
            f_t, den_t = sig_phase(1, 0, r_b)
            sc_b()
            r_c, sc_c = rec_phase(1, 0, f_t, den_t, None)
            f_t, den_t = sig_phase(1, 1, r_c)
            _, carry = sc_c(epilogue(0))
            r_d, sc_d = rec_phase(1, 1, f_t, den_t, carry)
            sc_d(epilogue(1))

            # ---- final LN + projection math on [8, 512] ----
            # y = -( (s1_0*A - s3_0) + Kc ) * rsqrt(v + eps') + c0
            # v  = (s2_0 + 2*sc)/D - ((s1_0 + C1)/D)^2
            sc_sb = fin.tile([NT, TT], F32, tag="fin")
            nc.scalar.activation(sc_sb[:], s13_ps[64:64 + NT, :], AF.Copy)
            s3_sb = fin.tile([NT, TT], F32, tag="fin")
            nc.scalar.activation(s3_sb[:], s13_ps[32:32 + NT, :], AF.Copy)
            s2c = fin.tile([NT, TT], F32, tag="fin")
            nc.vector.scalar_tensor_tensor(
                s2c[:], sc_sb[:], 2.0, s2_ps[:], OP.mult, OP.add)
            mu2 = fin.tile([NT, TT], F32, tag="fin")
            nc.scalar.activation(mu2[:], s13_ps[0:NT, :], AF.Square,
                                 bias=epi_sb[:, 3:4], scale=1.0 / D)
            v = fin.tile([NT, TT], F32, tag="fin")
            nc.vector.scalar_tensor_tensor(
                v[:], s2c[:], 1.0 / D, mu2[:], OP.mult, OP.subtract)
            rv = fin.tile([NT, TT], F32, tag="fin")
            _act_direct(nc, rv[:], v[:], AF.Rsqrt, bias=epi_sb[:, 2:3])
            q = fin.tile([NT, TT], F32, tag="fin")
            nc.vector.scalar_tensor_tensor(
                q[:], s13_ps[0:NT, :], epi_sb[:, 1:2], s3_sb[:],
                OP.mult, OP.subtract)
            z = fin.tile([NT, TT], F32, tag="fin")
            nc.vector.scalar_tensor_tensor(
                z[:], q[:], epi_sb[:, 4:5], rv[:], OP.add, OP.mult)
            o_sb = fin.tile([NT, TT], F32, tag="fin")
            nc.scalar.activation(o_sb[:], z[:], AF.Identity,
                                 bias=epi_sb[:, 0:1], scale=-1.0)
            nc.sync.dma_start(out=out_d[:], in_=o_sb[:])

    _split_excess_waits(nc)
    return nc


_NC_CACHE = None


def _get_nc():
    global _NC_CACHE
    if _NC_CACHE is None:
        _NC_CACHE = _build_nc()
    return _NC_CACHE


def _host_prep(inputs):
    x = np.asarray(inputs["x"], dtype=np.float32)
    Ws = [np.asarray(inputs[n], np.float32) for n in
          ("Wf0", "Wi0", "Wh0", "Wf1", "Wi1", "Wh1")]
    bs = [np.asarray(inputs[n], np.float32) for n in
          ("bf0", "bi0", "bh0", "bf1", "bi1", "bh1")]
    bf0, bi0, bh0, bf1, bi1, bh1 = bs
    Wf1, Wi1, Wh1 = Ws[3], Ws[4], Ws[5]
    # h-bias elimination: layer-2 gate biases absorb Wx1 @ bh0
    bf1e = bf1 + Wf1 @ bh0
    bi1e = bi1 + Wi1 @ bh0
    bh1e = bh1 + Wh1 @ bh0

    wt_all = np.ascontiguousarray(
        np.stack([w.T for w in Ws])).astype(ml_dtypes.bfloat16)  # [6, din, dout]

    bias = np.zeros((128, 2, 8), np.float32)
    gi = np.zeros((128, 2, G), np.float32)
    for g in range(G):
        sl = slice(g * 128, (g + 1) * 128)
        bias[:, 0, 0 * 4 + g] = bf0[sl]
        bias[:, 0, 1 * 4 + g] = bi0[sl]
        bias[:, 1, 0 * 4 + g] = bf1e[sl]
        bias[:, 1, 1 * 4 + g] = bi1e[sl]
        gi[:, 0, g] = -bh0[sl]
        gi[:, 1, g] = -bh1e[sl]
    # min-gate trick requires |bias| < 0.5 <= r = 1/(f+i)
    assert np.abs(bias).max() < 0.45, "bias magnitude breaks min-gate trick"

    w_out = np.asarray(inputs["W_out"], np.float32).reshape(D)
    ln_g = np.asarray(inputs["ln_g"], np.float32)
    ln_b = np.asarray(inputs["ln_b"], np.float32)
    b_out = float(np.asarray(inputs["b_out"], np.float32).reshape(()))
    wg = w_out * ln_g
    c = bh1e                         # constant channel shift of res
    c0 = float(w_out @ ln_b) + b_out
    swg = float(wg.sum())
    C1 = float(c.sum())
    C2 = float((c * c).sum())
    C3 = float((wg * c).sum())
    A = swg / D
    Kc = C1 * A - C3
    epsP = LN_EPS + C2 / D
    C1D = C1 / D

    slt = np.zeros((G, NT, 128, 72), np.float32)
    for g in range(G):
        sl = slice(g * 128, (g + 1) * 128)
        for t in range(NT):
            slt[g, t, :, t] = 1.0
            slt[g, t, :, 32 + t] = wg[sl]
            slt[g, t, :, 64 + t] = c[sl]
    slt = slt.astype(ml_dtypes.bfloat16)

    epi = np.zeros((NT, 8), np.float32)
    epi[:, 0] = c0
    epi[:, 1] = A
    epi[:, 2] = epsP
    epi[:, 3] = C1D
    epi[:, 4] = Kc

    xt_b = np.ascontiguousarray(
        x.transpose(0, 2, 1)).astype(ml_dtypes.bfloat16)   # [B, D, T]
    return xt_b, wt_all, bias, gi, slt, epi


def _in_maps(inputs):
    xt_b, wt_all, bias, gi, slt, epi = _host_prep(inputs)
    return [
        {
            "xt": xt_b[b],
            "wt": wt_all, "bias": bias, "gi": gi, "slt": slt, "epi": epi,
        }
        for b in range(B)
    ]


def kernel(**inputs):
    nc = _get_nc()
    res = run_bass_kernel_spmd(nc, _in_maps(inputs), list(range(B)))
    out = np.stack([res.results[b]["out"].reshape(T, OUT) for b in range(B)])
    return out.astype(np.float32)


def kernel_traced(**inputs):
    """same as kernel() but returns (output, BassKernelResults) with timing"""
    nc = _get_nc()
    res = run_bass_kernel_spmd(nc, _in_maps(inputs), list(range(B)), trace=True)
    out = np.stack([res.results[b]["out"].reshape(T, OUT) for b in range(B)])
    return out.astype(np.float32), res


# revision 14
# speedup vs baseline: 1.2897x; 1.0114x over previous
"""Trainium2 Bass kernel for nn_DecoderMinLSTMGNN.

Model (per sample): two MinLSTM layers (D=512) over T=4096 steps, residual,
LayerNorm, projection D->1.  B=8 samples are data-parallel across the 8
NeuronCores (one sample per core).

Per-core layout is channels-major: x^T [D, T], bf16.  The time-dim linear
recurrence h_t = a_t*h_{t-1} + (1-a_t)*htilde_t maps onto the VectorE
TensorTensorScan instruction (scan along the free dim).

Key optimizations over the naive structure:
- h-gate bias elimination: substitute g = h - bh.  The recurrence becomes
  bias-free (g_t = a_t g_{t-1} + (1-a_t) zh_t with zh = W_h x, init -bh);
  the bias reappears as a constant shift that folds into the next layer's
  f/i gate biases (bf1_eff = bf1 + Wf1 @ bh0) and into the LN/projection
  stats via an extra lhsT column + host-side constants.  Removes 64
  bias matmuls.
- ScalarE act-table phase batching: sigmoids and reciprocals use different
  act-table sets (~1.3us per table load; the naive interleaving costs ~97
  loads = 124us).  Work is organized in (layer, half-of-T) phases: all 32
  sigmoids of a phase, then all reciprocals.  A "min-gate" artificial
  dependency (phase biases are routed through tensor_tensor(min) against
  the previous phase's reciprocal output, exact because |bias|<0.45 and
  r=1/(f+i)>=0.5) forces the scheduler to keep phases contiguous:
  9 table loads total.
- bf16 operands everywhere (matmuls, DVE tensor-tensor 2x mode, half the
  SBUF/DMA footprint).  PSUM accumulation and scan state stay fp32.
- Wide [128, 2048] instructions for reciprocal/square/a/den/res/scan to
  amortize per-instruction overhead.
"""

import numpy as np
import ml_dtypes

import concourse.bass as bass
import concourse.mybir as mybir
import concourse.tile as tile
from concourse.bass_utils import run_bass_kernel_spmd

F32 = mybir.dt.float32
BF16 = mybir.dt.bfloat16
FP8 = mybir.dt.float8e4
DR = mybir.MatmulPerfMode.DoubleRow
WS = 64.0
AF = mybir.ActivationFunctionType
OP = mybir.AluOpType

B, T, D = 8, 4096, 512
OUT = 1
LN_EPS = 1e-5
TT = 512                 # time-tile size
NT = T // TT             # 8 time tiles
G = D // 128             # 4 channel groups
K = D // 128             # 4 contraction chunks
TPH = 4                  # time tiles per phase (half)
HALF = TPH * TT          # 2048
NH = NT // TPH           # 2 halves

MAX_WAITS = 1


def _split_excess_waits(nc):
    """walrus in this container rejects >1 semaphore wait per instruction
    ("Too many sync wait commands"); move excess waits onto NoOps."""
    for fn in nc.m.functions:
        for bb in fn.blocks:
            new_list = []
            changed = False
            for inst in bb.instructions:
                si = inst.sync_info
                waits = list(si.on_wait) if si is not None and si.on_wait else []
                if len(waits) > MAX_WAITS:
                    changed = True
                    overflow = waits[:-MAX_WAITS]
                    si.on_wait = waits[-MAX_WAITS:]
                    for j in range(0, len(overflow), MAX_WAITS):
                        new_list.append(mybir.InstNoOp(
                            name=f"{inst.name}-waitsplit-{j}",
                            engine=inst.engine,
                            ins=[], outs=[],
                            sync_info=mybir.SyncInfo(
                                on_wait=overflow[j:j + MAX_WAITS], on_update=[]),
                        ))
                new_list.append(inst)
            if changed:
                bb.instructions[:] = new_list
    return nc


def _act_direct(nc, out, in_, func, bias=0.0, scale=1.0):
    """emit InstActivation directly (bass blocks Reciprocal/Rsqrt)."""
    ins = [nc.scalar.lower_ap(in_)]
    for v in (bias, scale, 0.0):
        if isinstance(v, (int, float)):
            ins.append(mybir.ImmediateValue(dtype=mybir.dt.float32, value=float(v)))
        else:
            ins.append(nc.scalar.lower_ap(v))
    return nc.scalar.add_instruction(
        mybir.InstActivation(
            name=nc.get_next_instruction_name(),
            func=func, ins=ins, outs=[nc.scalar.lower_ap(out)]))


def _build_nc():
    nc = bass.Bass()

    xt_d = nc.dram_tensor("xt", [D, T], BF16, kind="ExternalInput")
    # fp8 x for the f/i gate matmuls (DoubleRow): [c, p, i, t], ch = c*256+i*128+p
    xf8_d = nc.dram_tensor("xf8", [2, 128, 2, T], FP8, kind="ExternalInput")
    # fp8 f/i weights (x WS), [layer*2+gate, p, c, i, dout]
    wfi_d = nc.dram_tensor("wfi", [4, 128, 2, 2, D], FP8, kind="ExternalInput")
    # bf16 h-gate weights only: [layer, din, dout]
    wt_d = nc.dram_tensor("wt", [2, D, D], BF16, kind="ExternalInput")
    # f/i gate biases (layer-2 ones pre-corrected): [128, layer, gate*4+g]
    bias_d = nc.dram_tensor("bias", [128, 2, 8], F32, kind="ExternalInput")
    # scan initial state columns (-bh_eff): [128, layer, g]
    gi_d = nc.dram_tensor("gi", [128, 2, G], F32, kind="ExternalInput")
    # stats lhsT per (g,t): col t = 1 (s1), col 32+t = wg, col 64+t = c
    slt_d = nc.dram_tensor("slt", [G, NT, 128, 72], BF16, kind="ExternalInput")
    epi_d = nc.dram_tensor("epi", [NT, 8], F32, kind="ExternalInput")
    out_d = nc.dram_tensor("out", [NT, TT], F32, kind="ExternalOutput")

    with tile.TileContext(nc) as tc:
        with (
            tc.tile_pool(name="const", bufs=1) as const,
            tc.tile_pool(name="xtp", bufs=1) as xtp,
            tc.tile_pool(name="gp", bufs=1) as gp,        # wide per-g phase bufs
            tc.tile_pool(name="work", bufs=2) as work,    # i tiles
            tc.tile_pool(name="wk2", bufs=2) as wk2,      # res/sq/bgate/carry
            tc.tile_pool(name="fin", bufs=4) as fin,
            tc.tile_pool(name="gates_ps", bufs=6, space="PSUM") as gates_ps,
            tc.tile_pool(name="stats_ps", bufs=1, space="PSUM") as stats_ps,
        ):
            # ---- constants + x, DMA-ordered so phase (0,0) starts ASAP ----
            wt_sb = [None] * 2
            wfi_sb = [None] * 4
            def _load_wh(idx):
                w = const.tile([128, K, D], BF16, tag=f"wt{idx}", name=f"wt{idx}")
                nc.sync.dma_start(
                    out=w[:], in_=wt_d[idx].rearrange("(k p) d -> p k d", p=128))
                wt_sb[idx] = w
            def _load_wfi(idx):
                w = const.tile([128, 2, 2, D], FP8, tag=f"wfi{idx}", name=f"wfi{idx}")
                nc.sync.dma_start(out=w[:], in_=wfi_d[idx])
                wfi_sb[idx] = w
            bias_sb = const.tile([128, 2, 8], F32)
            nc.sync.dma_start(out=bias_sb[:], in_=bias_d[:])
            _load_wfi(0)
            _load_wfi(1)
            xf8_sb = []
            for c in range(2):
                xc = xtp.tile([128, 2, T], FP8, tag=f"xf8{c}", name=f"xf8{c}")
                nc.gpsimd.dma_start(out=xc[:], in_=xf8_d[c])
                xf8_sb.append(xc)
            xt_sb = []
            for g in range(G):
                xx = xtp.tile([128, T], BF16, tag=f"xt{g}", name=f"xt{g}")
                xt_sb.append(xx)
            for h in range(NH):
                for g in range(G):
                    nc.gpsimd.dma_start(
                        out=xt_sb[g][:, h * HALF:(h + 1) * HALF],
                        in_=xt_d[g * 128:(g + 1) * 128, h * HALF:(h + 1) * HALF])
            _load_wh(0)
            gi_sb = const.tile([128, 2, G], F32)
            nc.sync.dma_start(out=gi_sb[:], in_=gi_d[:])
            _load_wfi(2)
            _load_wfi(3)
            _load_wh(1)
            slt_sb = const.tile([128, G, NT, 72], BF16)
            nc.sync.dma_start(
                out=slt_sb[:], in_=slt_d.rearrange("g t p c -> p g t c"))
            epi_sb = const.tile([NT, 8], F32)
            nc.sync.dma_start(out=epi_sb[:], in_=epi_d[:])
            # fp8 copies of g1 for the layer-2 f/i matmuls; alias the xf8
            # slots (xf8 is dead after the last layer-0 f/i matmul).
            gf8_sb = [None, None]

            # layer-1 scan outputs (bf16), resident per (g, half)
            g1_sb = [[None] * NH for _ in range(G)]
            # persistent stats accumulators (PSUM)
            s13_ps = stats_ps.tile([72, TT], F32, tag="s13")
            s2_ps = stats_ps.tile([NT, TT], F32, tag="s2")
            stats_first = [True]
            stats_count = [0]
            N_STATS = G * NT         # stats matmul pairs = 32

            def sig_phase(layer, half, gate_r):
                """all zf/zi matmuls + sigmoids + den for one (layer, half).
                gate_r: previous phase's reciprocal tile (or None) used to
                gate this phase's biases (forces ScalarE phase ordering)."""
                if gate_r is None:
                    bsrc = lambda gate, g: bias_sb[:, layer, 4 * gate + g:4 * gate + g + 1]
                else:
                    bg = wk2.tile([128, 8], F32, tag="bgate")
                    nc.vector.tensor_tensor(
                        bg[:], bias_sb[:, layer], gate_r[0][:, 0:8], OP.min)
                    for rr in gate_r[1:]:
                        bg2 = wk2.tile([128, 8], F32, tag="bgate", name="bg2")
                        nc.vector.tensor_tensor(bg2[:], bg[:], rr[:, 0:8], OP.min)
                        bg = bg2
                    bsrc = lambda gate, g: bg[:, 4 * gate + g:4 * gate + g + 1]
                rhs = (xt_sb if layer == 0 else None)
                f_t, den_t = [], []
                for g in range(G):
                    f_t.append(gp.tile([128, HALF], BF16, tag=f"fr{g}", bufs=2, name=f"f{g}"))
                    den_t.append(gp.tile([128, HALF], BF16, tag=f"da{g}", bufs=2, name=f"den{g}"))
                src8 = xf8_sb if layer == 0 else gf8_sb
                for ti in range(TPH):
                    t = half * TPH + ti
                    for g in range(G):
                        pf = gates_ps.tile([128, TT], F32, tag="mm", name="pf")
                        pi = gates_ps.tile([128, TT], F32, tag="mm", name="pi")
                        for gate, ps in ((0, pf), (1, pi)):
                            w8 = wfi_sb[2 * layer + gate]
                            for c in range(2):
                                nc.tensor.matmul(
                                    ps[:], w8[:, c, :, g * 128:(g + 1) * 128],
                                    src8[c][:, :, t * TT:(t + 1) * TT],
                                    start=(c == 0), stop=(c == 1), perf_mode=DR)
                        fs = f_t[g][:, ti * TT:(ti + 1) * TT]
                        nc.scalar.activation(fs, pf[:], AF.Sigmoid, bias=bsrc(0, g),
                                             scale=1.0 / WS)
                        i_sb = work.tile([128, TT], BF16, tag="i")
                        nc.scalar.activation(i_sb[:], pi[:], AF.Sigmoid, bias=bsrc(1, g),
                                             scale=1.0 / WS)
                        nc.vector.tensor_add(
                            den_t[g][:, ti * TT:(ti + 1) * TT], fs, i_sb[:])
                        i_last = i_sb
                # gate all reciprocals on the last sigmoid of the phase:
                # max(den, i) == den exactly (den = f+i >= i), so this only
                # adds the dependency, keeping the act-table phases contiguous.
                for g in range(G):
                    nc.vector.tensor_tensor(
                        den_t[g][:, 0:1], den_t[g][:, 0:1], i_last[:, 0:1], OP.max)
                return f_t, den_t

            def rec_phase(layer, half, f_t, den_t, g2_carry):
                """reciprocal + a + u' for one (layer, half); scans are
                emitted later (emit_scans closure) so the next sig phase's
                den adds aren't queued behind them on DVE."""
                r_t, a_t, up_t = [], [], []
                for g in range(G):
                    r = gp.tile([128, HALF], BF16, tag=f"fr{g}", bufs=2, name=f"r{g}")
                    _act_direct(nc, r[:], den_t[g][:], AF.Reciprocal)
                    r_t.append(r)
                for g in range(G):
                    a = gp.tile([128, HALF], BF16, tag=f"da{g}", bufs=2, name=f"a{g}")
                    nc.vector.tensor_mul(a[:], f_t[g][:], r_t[g][:])
                    a_t.append(a)
                    up_t.append(gp.tile([128, HALF], BF16, tag=f"up{g}", bufs=1, name=f"up{g}"))
                for ti in range(TPH):
                    t = half * TPH + ti
                    for g in range(G):
                        ph = gates_ps.tile([128, TT], F32, tag="mm", name="ph")
                        w = wt_sb[layer]
                        for k in range(K):
                            if layer == 0:
                                r = xt_sb[k][:, t * TT:(t + 1) * TT]
                            else:
                                r = g1_sb[k][half][:, ti * TT:(ti + 1) * TT]
                            nc.tensor.matmul(
                                ph[:], w[:, k, g * 128:(g + 1) * 128], r,
                                start=(k == 0), stop=(k == K - 1))
                        nc.vector.scalar_tensor_tensor(
                            up_t[g][:, ti * TT:(ti + 1) * TT],
                            a_t[g][:, ti * TT:(ti + 1) * TT], 1.0, ph[:],
                            OP.subtract, OP.mult)
                def emit_scans(epi_g=None):
                    gout = []
                    carry = [] if (layer == 1 and half == 0) else None
                    for g in range(G):
                        if layer == 0:
                            go = gp.tile([128, HALF], BF16, tag=f"g1_{g}_{half}", name=f"g1_{g}_{half}")
                            init = (gi_sb[:, 0, g:g + 1] if half == 0
                                    else g1_sb[g][0][:, HALF - 1:HALF])
                            g1_sb[g][half] = go
                        else:
                            go = gp.tile([128, HALF], BF16, tag=f"g1_{g}_0", name=f"g2_{g}_{half}")
                            init = (gi_sb[:, 1, g:g + 1] if half == 0
                                    else g2_carry[g][:])
                        nc.vector.tensor_tensor_scan(
                            go[:], a_t[g][:], up_t[g][:], init, OP.mult, OP.subtract)
                        gout.append(go)
                        if layer == 0:
                            c8, i8 = g // 2, g % 2
                            if gf8_sb[c8] is None:
                                gf8_sb[c8] = xtp.tile(
                                    [128, 2, T], FP8, tag=f"xf8{c8}", name=f"gf8{c8}")
                            nc.scalar.activation(
                                gf8_sb[c8][:, i8, half * HALF:(half + 1) * HALF],
                                go[:], AF.Copy)
                        if carry is not None:
                            cr = wk2.tile([128, 1], BF16, tag=f"carry{g}", name=f"carry{g}")
                            nc.vector.tensor_copy(cr[:], go[:, HALF - 1:HALF])
                            carry.append(cr)
                        if epi_g is not None:
                            epi_g(g, go)
                    return gout, carry
                return r_t, emit_scans

            def epilogue(half):
                """per-g closure: res = g2 + x^T, square, stats matmuls."""
                def epi_g(g, g2):
                    res = wk2.tile([128, HALF], BF16, tag="res", bufs=1)
                    nc.vector.tensor_add(
                        res[:], g2[:],
                        xt_sb[g][:, half * HALF:(half + 1) * HALF])
                    sq = wk2.tile([128, HALF], BF16, tag="sq", bufs=1)
                    nc.scalar.activation(sq[:], res[:], AF.Square)
                    for ti in range(TPH):
                        t = half * TPH + ti
                        rs = res[:, ti * TT:(ti + 1) * TT]
                        sqs = sq[:, ti * TT:(ti + 1) * TT]
                        first = stats_first[0]
                        stats_first[0] = False
                        stats_count[0] += 1
                        last = stats_count[0] == N_STATS
                        nc.tensor.matmul(
                            s13_ps[:], slt_sb[:, g, t, 0:72], rs,
                            start=first, stop=last, skip_group_check=True)
                        nc.tensor.matmul(
                            s2_ps[:], slt_sb[:, g, t, 0:8], sqs,
                            start=first, stop=last, skip_group_check=True)
                return epi_g

            # ---- pipeline (scans deferred past the next sig phase) ----
            f_t, den_t = sig_phase(0, 0, None)
            r_a, sc_a = rec_phase(0, 0, f_t, den_t, None)
            f_t, den_t = sig_phase(0, 1, r_a)
            sc_a()
            r_b, sc_b = rec_phase(0, 1, f_t, den_t, None)
            f_t, den_t = sig_phase(1, 0, r_b)
            sc_b()
            r_c, sc_c = rec_phase(1, 0, f_t, den_t, None)
            f_t, den_t = sig_phase(1, 1, r_c)
            _, carry = sc_c(epilogue(0))
            r_d, sc_d = rec_phase(1, 1, f_t, den_t, carry)
            sc_d(epilogue(1))

            # ---- final LN + projection math on [8, 512] ----
            # y = -( (s1_0*A - s3_0) + Kc ) * rsqrt(v + eps') + c0
            # v  = (s2_0 + 2*sc)/D - ((s1_0 + C1)/D)^2
            sc_sb = fin.tile([NT, TT], F32, tag="fin")
            nc.scalar.activation(sc_sb[:], s13_ps[64:64 + NT, :], AF.Copy)
            s3_sb = fin.tile([NT, TT], F32, tag="fin")
            nc.scalar.activation(s3_sb[:], s13_ps[32:32 + NT, :], AF.Copy)
            s2c = fin.tile([NT, TT], F32, tag="fin")
            nc.vector.scalar_tensor_tensor(
                s2c[:], sc_sb[:], 2.0, s2_ps[:], OP.mult, OP.add)
            mu2 = fin.tile([NT, TT], F32, tag="fin")
            nc.scalar.activation(mu2[:], s13_ps[0:NT, :], AF.Square,
                                 bias=epi_sb[:, 3:4], scale=1.0 / D)
            v = fin.tile([NT, TT], F32, tag="fin")
            nc.vector.scalar_tensor_tensor(
                v[:], s2c[:], 1.0 / D, mu2[:], OP.mult, OP.subtract)
            rv = fin.tile([NT, TT], F32, tag="fin")
            _act_direct(nc, rv[:], v[:], AF.Rsqrt, bias=epi_sb[:, 2:3])
            q = fin.tile([NT, TT], F32, tag="fin")
            nc.vector.scalar_tensor_tensor(
                q[:], s13_ps[0:NT, :], epi_sb[:, 1:2], s3_sb[:],
                OP.mult, OP.subtract)
            z = fin.tile([NT, TT], F32, tag="fin")
            nc.vector.scalar_tensor_tensor(
                z[:], q[:], epi_sb[:, 4:5], rv[:], OP.add, OP.mult)
            o_sb = fin.tile([NT, TT], F32, tag="fin")
            nc.scalar.activation(o_sb[:], z[:], AF.Identity,
                                 bias=epi_sb[:, 0:1], scale=-1.0)
            nc.sync.dma_start(out=out_d[:], in_=o_sb[:])

    _split_excess_waits(nc)
    return nc


_NC_CACHE = None


def _get_nc():
    global _NC_CACHE
    if _NC_CACHE is None:
        _NC_CACHE = _build_nc()
    return _NC_CACHE


def _host_prep(inputs):
    x = np.asarray(inputs["x"], dtype=np.float32)
    Ws = [np.asarray(inputs[n], np.float32) for n in
          ("Wf0", "Wi0", "Wh0", "Wf1", "Wi1", "Wh1")]
    bs = [np.asarray(inputs[n], np.float32) for n in
          ("bf0", "bi0", "bh0", "bf1", "bi1", "bh1")]
    bf0, bi0, bh0, bf1, bi1, bh1 = bs
    Wf1, Wi1, Wh1 = Ws[3], Ws[4], Ws[5]
    # h-bias elimination: layer-2 gate biases absorb Wx1 @ bh0
    bf1e = bf1 + Wf1 @ bh0
    bi1e = bi1 + Wi1 @ bh0
    bh1e = bh1 + Wh1 @ bh0

    # bf16 h-gate weights only
    wt_all = np.ascontiguousarray(
        np.stack([Ws[2].T, Ws[5].T])).astype(ml_dtypes.bfloat16)  # [2, din, dout]
    # fp8 f/i weights, scaled by WS (undone via the sigmoid input scale) to
    # keep them out of the fp8 subnormal range; [l*2+gate, p, c, i, dout]
    np_fp8 = mybir.dt.np(mybir.dt.float8e4)
    wfi = np.stack([
        (Ws[j].T * WS).reshape(2, 2, 128, D).transpose(2, 0, 1, 3)
        for j in (0, 1, 3, 4)
    ]).astype(np_fp8)

    bias = np.zeros((128, 2, 8), np.float32)
    gi = np.zeros((128, 2, G), np.float32)
    for g in range(G):
        sl = slice(g * 128, (g + 1) * 128)
        bias[:, 0, 0 * 4 + g] = bf0[sl]
        bias[:, 0, 1 * 4 + g] = bi0[sl]
        bias[:, 1, 0 * 4 + g] = bf1e[sl]
        bias[:, 1, 1 * 4 + g] = bi1e[sl]
        gi[:, 0, g] = -bh0[sl]
        gi[:, 1, g] = -bh1e[sl]
    # min-gate trick requires |bias| < 0.5 <= r = 1/(f+i)
    assert np.abs(bias).max() < 0.45, "bias magnitude breaks min-gate trick"

    w_out = np.asarray(inputs["W_out"], np.float32).reshape(D)
    ln_g = np.asarray(inputs["ln_g"], np.float32)
    ln_b = np.asarray(inputs["ln_b"], np.float32)
    b_out = float(np.asarray(inputs["b_out"], np.float32).reshape(()))
    wg = w_out * ln_g
    c = bh1e                         # constant channel shift of res
    c0 = float(w_out @ ln_b) + b_out
    swg = float(wg.sum())
    C1 = float(c.sum())
    C2 = float((c * c).sum())
    C3 = float((wg * c).sum())
    A = swg / D
    Kc = C1 * A - C3
    epsP = LN_EPS + C2 / D
    C1D = C1 / D

    slt = np.zeros((G, NT, 128, 72), np.float32)
    for g in range(G):
        sl = slice(g * 128, (g + 1) * 128)
        for t in range(NT):
            slt[g, t, :, t] = 1.0
            slt[g, t, :, 32 + t] = wg[sl]
            slt[g, t, :, 64 + t] = c[sl]
    slt = slt.astype(ml_dtypes.bfloat16)

    epi = np.zeros((NT, 8), np.float32)
    epi[:, 0] = c0
    epi[:, 1] = A
    epi[:, 2] = epsP
    epi[:, 3] = C1D
    epi[:, 4] = Kc

    xt = x.transpose(0, 2, 1)                              # [B, D, T]
    xt_b = np.ascontiguousarray(xt).astype(ml_dtypes.bfloat16)
    # fp8 x chunks for DoubleRow: [B, c, p, i, t], channel = c*256+i*128+p
    xf8 = np.ascontiguousarray(
        xt.reshape(B, 2, 2, 128, T).transpose(0, 1, 3, 2, 4)).astype(np_fp8)
    return xt_b, xf8, wt_all, wfi, bias, gi, slt, epi


def _in_maps(inputs):
    xt_b, xf8, wt_all, wfi, bias, gi, slt, epi = _host_prep(inputs)
    return [
        {
            "xt": xt_b[b], "xf8": xf8[b],
            "wt": wt_all, "wfi": wfi, "bias": bias, "gi": gi,
            "slt": slt, "epi": epi,
        }
        for b in range(B)
    ]


def kernel(**inputs):
    nc = _get_nc()
    res = run_bass_kernel_spmd(nc, _in_maps(inputs), list(range(B)))
    out = np.stack([res.results[b]["out"].reshape(T, OUT) for b in range(B)])
    return out.astype(np.float32)


def kernel_traced(**inputs):
    """same as kernel() but returns (output, BassKernelResults) with timing"""
    nc = _get_nc()
    res = run_bass_kernel_spmd(nc, _in_maps(inputs), list(range(B)), trace=True)
    out = np.stack([res.results[b]["out"].reshape(T, OUT) for b in range(B)])
    return out.astype(np.float32), res
